# revision 1
# baseline (speedup 1.0000x reference)
"""Trainium2 Bass kernel for multi-head attention (B=4, L=2048, D=1024, H=16).

Sharding: 8 cores = 4 batches x 2 head-groups (8 heads each).
Per core: QKV projection (its head slice), RoPE, per-head attention
(scores stored transposed [k,q] so the softmax denominator folds into the
PV matmul via a ones-column on V), output projection against its w_out
column slice.  Host sums the two per-batch partials (tensor-parallel
reduce done on host since full output must be gathered anyway).

All matmul operands are bf16 (fp32 PSUM accumulation); output fp32.
"""
import sys

sys.path.insert(0, "/opt/trn_rl_repo")
import numpy as np
import concourse.bass as bass
import concourse.bacc as bacc
import concourse.mybir as mybir
from concourse.tile import TileContext
from concourse.bass_utils import run_bass_kernel_spmd

L = 2048          # sequence length
D = 1024          # model dim
HD = 64           # head dim
NH_CORE = 8       # heads per core
F_QK = 1024       # q+k features per core
F_V = 512         # v features per core
KT = L // 128     # 16 k position tiles
QC = 4            # q chunks of 512
DT = mybir.dt.bfloat16
F32 = mybir.dt.float32
SCALE = HD ** -0.5
AF = mybir.ActivationFunctionType


def build_nc():
    nc = bacc.Bacc("TRN2", target_bir_lowering=False, debug=False, num_devices=8)
    xT = nc.dram_tensor("xT", [D, L], DT, kind="ExternalInput")
    wqkT = nc.dram_tensor("wqkT", [8, 128, 8, 128], DT, kind="ExternalInput")
    wvT = nc.dram_tensor("wvT", [D, F_V], DT, kind="ExternalInput")
    bqk = nc.dram_tensor("bqk", [128, 8], F32, kind="ExternalInput")
    bv = nc.dram_tensor("bv", [1, F_V], DT, kind="ExternalInput")
    woT = nc.dram_tensor("woT", [F_V, D], DT, kind="ExternalInput")
    bout = nc.dram_tensor("bout", [1, D], DT, kind="ExternalInput")
    cosT = nc.dram_tensor("cosT", [128, L], DT, kind="ExternalInput")
    sinT = nc.dram_tensor("sinT", [128, L], DT, kind="ExternalInput")
    out = nc.dram_tensor("out", [L, D], F32, kind="ExternalOutput")

    with TileContext(nc) as tc:
        with (
            tc.tile_pool(name="const", bufs=1) as cp,
            tc.tile_pool(name="wstream", bufs=2) as wsp,
            tc.tile_pool(name="rope", bufs=2) as rp,
            tc.tile_pool(name="exps", bufs=2) as ep,
            tc.tile_pool(name="ctile", bufs=2) as ctp,
            tc.tile_pool(name="small", bufs=4) as sp,
            tc.tile_pool(name="psum", bufs=1, space="PSUM") as pp,
        ):
            dma = nc.default_dma_engine

            # ---- resident inputs ----
            # chunk-interleaved so the k=0 operands of the first matmuls land first
            xT_sb = cp.tile([128, 8, L], DT)        # x.T  [d-chunk partitions, chunk, l]
            wvT_sb = cp.tile([128, 8, F_V], DT)
            wqk_tiles = {}
            dma2 = nc.gpsimd
            for c in range(8):
                dma2.dma_start(out=wvT_sb[:, c, :], in_=wvT[c * 128:(c + 1) * 128, :])
                (dma, dma2)[c % 2].dma_start(out=xT_sb[:, c, :],
                                             in_=xT[c * 128:(c + 1) * 128, :])
                if c < 2:   # prefetch first head-pair's projection weights early
                    fc = (0, 4)[c]
                    wqk_tiles[fc] = wsp.tile([128, 8, 128], DT, tag="wqk",
                                             name=f"wqk{fc}")
                    dma.dma_start(out=wqk_tiles[fc][:], in_=wqkT[fc])
            woT_sb = cp.tile([128, 4, D], DT)
            for c in range(4):
                dma.dma_start(out=woT_sb[:, c, :], in_=woT[c * 128:(c + 1) * 128, :])
            cos_sb = cp.tile([128, L], DT)
            dma.dma_start(out=cos_sb[:], in_=cosT[:])
            sin_sb = cp.tile([128, L], DT)
            dma.dma_start(out=sin_sb[:], in_=sinT[:])
            bqk_sb = cp.tile([128, 8], F32)
            dma.dma_start(out=bqk_sb[:], in_=bqk[:])
            bv_sb = cp.tile([1, F_V], DT)
            dma.dma_start(out=bv_sb[:], in_=bv[:])
            bout_sb = cp.tile([1, D], DT)
            dma.dma_start(out=bout_sb[:], in_=bout[:])
            bv_bc = cp.tile([128, F_V], DT)
            nc.gpsimd.partition_broadcast(bv_bc[:], bv_sb[:])
            bout_bc = cp.tile([128, D], DT)
            nc.gpsimd.partition_broadcast(bout_bc[:], bout_sb[:])

            qkT = cp.tile([128, 8, L], DT)          # q (chunks 0-3) / k (chunks 4-7), feature-major
            V_sb = cp.tile([128, KT, 8 * (HD + 1)], DT)  # position-major V + ones col per head

            # ---- projection + RoPE helpers (emitted lazily, see stream order) ----
            def qk_proj(fc):
                if fc in wqk_tiles:
                    wqk_t = wqk_tiles.pop(fc)
                else:
                    wqk_t = wsp.tile([128, 8, 128], DT, tag="wqk", name=f"wqk{fc}")
                    dma.dma_start(out=wqk_t[:], in_=wqkT[fc])
                for nt in range(4):
                    tag, bufs = (("sA", 1), ("ot", 2), ("sB", 1), ("ot", 2))[nt]
                    qps = pp.tile([128, 512], F32, tag=tag, bufs=bufs,
                                  name=f"qps{fc}_{nt}")
                    for kc in range(8):
                        nc.tensor.matmul(qps[:], lhsT=wqk_t[:, kc, :],
                                         rhs=xT_sb[:, kc, nt * 512:(nt + 1) * 512],
                                         start=(kc == 0), stop=(kc == 7))
                    nc.vector.tensor_scalar_add(qkT[:, fc, nt * 512:(nt + 1) * 512],
                                                qps[:], bqk_sb[:, fc:fc + 1])

            def rope(c):
                # layout per 128-partition chunk: 2 heads x (half0 32, half1 32)
                rot = rp.tile([128, L], DT, tag="rot", name=f"rot{c}")
                for h2 in range(2):
                    p = 64 * h2
                    dma.dma_start(out=rot[p:p + 32, :], in_=qkT[p + 32:p + 64, c, :])
                    dma.dma_start(out=rot[p + 32:p + 64, :], in_=qkT[p:p + 32, c, :])
                tmp = rp.tile([128, L], DT, tag="ropetmp", name=f"ropetmp{c}")
                nc.vector.tensor_mul(tmp[:], qkT[:, c, :], cos_sb[:])
                nc.vector.tensor_mul(rot[:], rot[:], sin_sb[:])
                nc.vector.tensor_add(qkT[:, c, :], tmp[:], rot[:])

            def v_proj(lt):
                tag, bufs = (("sA", 1), ("ot", 2), ("sB", 1), ("ot", 2))[lt % 4]
                vps = pp.tile([128, 512], F32, tag=tag, bufs=bufs, name=f"vps{lt}")
                for kc in range(8):
                    nc.tensor.matmul(vps[:],
                                     lhsT=xT_sb[:, kc, lt * 128:(lt + 1) * 128],
                                     rhs=wvT_sb[:, kc, :], start=(kc == 0), stop=(kc == 7))
                v4 = V_sb[:, lt, :].rearrange("p (h c) -> p h c", c=HD + 1)
                nc.vector.tensor_add(
                    v4[:, :, 0:HD],
                    vps[:].rearrange("p (h c) -> p h c", c=HD),
                    bv_bc[:].rearrange("p (h c) -> p h c", c=HD))
                nc.vector.memset(v4[:, :, HD:HD + 1], 1.0)

            for lt in range(KT):
                v_proj(lt)
            for fc in (0, 4, 1, 5, 2, 6, 3, 7):
                qk_proj(fc)
            for c in (0, 4, 1, 5, 2, 6, 3, 7):
                rope(c)

            # ---- phase 2: attention + output projection ----
            # Both heads of a pair run together: their S.T matmuls contract
            # K=64 from partitions 0-63 / 64-127, i.e. different PE row
            # groups, so adjacent matmuls overlap in the array on HW.
            # The very first (qc=0, hp=0) pass interleaves the V projection
            # into its PV stream (PV of k-tile kt only needs V tile lt=kt);
            # later head-pairs' qk projections + RoPE are emitted just
            # before their first use.
            pending_op = []
            for qc in range(QC):
                cT = ctp.tile([128, 4, 512], DT, tag="cT", name=f"cT{qc}")
                for hp in range(4):
                    for _ in range(2):
                        if pending_op:
                            pending_op.pop(0)()
                    expA = ep.tile([128, KT, 512], DT, tag="expA", bufs=1)
                    expB = ep.tile([128, KT, 512], DT, tag="expB", bufs=1)
                    otA = pp.tile([128, 512], F32, tag="ot", bufs=2)
                    otB = pp.tile([128, 512], F32, tag="ot", bufs=2)

                    def pv_tiles(kts):
                        for kt in kts:
                            for h2, expS, ot in ((0, expA, otA), (1, expB, otB)):
                                h = 2 * hp + h2
                                nc.tensor.matmul(
                                    ot[0:65, :],
                                    lhsT=V_sb[:, kt, h * 65:(h + 1) * 65],
                                    rhs=expS[:, kt, :],
                                    start=(kt == 0), stop=(kt == KT - 1))

                    # k-tile groups of 3 (then 2,2): exp overhead amortizes
                    # over [128, n*512]; A/B single-buffered 3-bank tiles.
                    groups = [(0, 1), (2, 3, 4), (5, 6, 7), (8, 9, 10),
                              (11, 12, 13), (14, 15)]
                    prev = None
                    for kts in groups:
                        n = len(kts)
                        spsA = pp.tile([128, 1536], F32, tag="sA", bufs=1)
                        spsB = pp.tile([128, 1536], F32, tag="sB", bufs=1)
                        for j, kt in enumerate(kts):
                            for p, sps in ((0, spsA), (64, spsB)):
                                nc.tensor.matmul(
                                    sps[:, j * 512:(j + 1) * 512],
                                    lhsT=qkT[p:p + 64, 4 + hp, kt * 128:(kt + 1) * 128],
                                    rhs=qkT[p:p + 64, hp, qc * 512:(qc + 1) * 512],
                                    start=True, stop=True)
                        nc.scalar.activation(
                            expA[:, kts[0]:kts[0] + n, :].rearrange("p a b -> p (a b)"),
                            spsA[:, 0:n * 512], AF.Exp, scale=SCALE)
                        nc.scalar.activation(
                            expB[:, kts[0]:kts[0] + n, :].rearrange("p a b -> p (a b)"),
                            spsB[:, 0:n * 512], AF.Exp, scale=SCALE)
                        if prev is not None:
                            pv_tiles(prev)
                        prev = kts
                    pv_tiles(prev)
                    for h2, ot in ((0, otA), (1, otB)):
                        rrow = sp.tile([1, 512], F32, tag="rrow")
                        nc.vector.reciprocal(rrow[:], ot[64:65, :])
                        bc = sp.tile([64, 512], F32, tag="bc")
                        nc.gpsimd.partition_broadcast(bc[:], rrow[:])
                        nc.vector.tensor_mul(cT[64 * h2:64 * h2 + 64, hp, :],
                                             ot[0:64, :], bc[:])
                # output projection groups for this q chunk; emitted into the
                # NEXT qc's stream (fills the PE bubbles of the ACT-bound
                # attention loop).  qc==3 flushes at the end.
                def op_group(qc, cT, dt_, mq):
                    def emit():
                        ops = pp.tile([128, 512], F32, tag="ot", bufs=2,
                                      name=f"ops{qc}_{dt_}_{mq}")
                        for cc in range(4):
                            nc.tensor.matmul(ops[:],
                                             lhsT=cT[:, cc, mq * 128:(mq + 1) * 128],
                                             rhs=woT_sb[:, cc, dt_ * 512:(dt_ + 1) * 512],
                                             start=(cc == 0), stop=(cc == 3))
                        osb = ctp.tile([128, 512], F32, tag="osb", bufs=4,
                                       name=f"osb{qc}_{dt_}_{mq}")
                        nc.vector.tensor_add(osb[:], ops[:],
                                             bout_bc[:, dt_ * 512:(dt_ + 1) * 512])
                        dma.dma_start(
                            out=out[qc * 512 + mq * 128: qc * 512 + (mq + 1) * 128,
                                    dt_ * 512:(dt_ + 1) * 512],
                            in_=osb[:])
                    return emit
                pending_op.extend(op_group(qc, cT, dt_, mq)
                                  for dt_ in range(2) for mq in range(4))
            for emit in pending_op:
                emit()
    nc.compile()
    return nc


def _rope_tables_np():
    inv_freq = 1.0 / (10000.0 ** (np.arange(0, HD, 2, dtype=np.float32) / HD))
    t = np.arange(L, dtype=np.float32)
    freqs = np.outer(t, inv_freq).astype(np.float32)       # [L, 32]
    cos_h = np.cos(freqs).T                                # [32, L]
    sin_h = np.sin(freqs).T
    cosT = np.concatenate([cos_h, cos_h], 0)               # [64, L]
    sinT = np.concatenate([-sin_h, sin_h], 0)              # sign baked for rot trick
    return np.tile(cosT, (2, 1)), np.tile(sinT, (2, 1))    # [128, L] (2 heads/tile)


_NC_CACHE = {}


def kernel(x, w_qkv, b_qkv, w_out, b_out):
    import ml_dtypes
    bf16 = ml_dtypes.bfloat16
    if "nc" not in _NC_CACHE:
        _NC_CACHE["nc"] = build_nc()
    nc = _NC_CACHE["nc"]

    cosT, sinT = _rope_tables_np()
    cosT = cosT.astype(bf16)
    sinT = sinT.astype(bf16)
    in_maps = []
    for c in range(8):
        b, g = divmod(c, 2)
        s = slice(512 * g, 512 * (g + 1))
        wqk = np.concatenate([w_qkv[0:D][s], w_qkv[D:2 * D][s]], 0)  # [1024, 1024]
        in_maps.append({
            "xT": np.ascontiguousarray(x[b].T).astype(bf16),
            "wqkT": np.ascontiguousarray(
                wqk.T.reshape(8, 128, 8, 128).transpose(2, 1, 0, 3)).astype(bf16),
            "wvT": np.ascontiguousarray(w_qkv[2 * D:3 * D][s].T).astype(bf16),
            "bqk": np.ascontiguousarray(
                np.concatenate([b_qkv[0:D][s], b_qkv[D:2 * D][s]])
                .reshape(8, 128).T).astype(np.float32),
            "bv": b_qkv[2 * D:3 * D][s][None].astype(bf16),
            "woT": np.ascontiguousarray(w_out[:, s].T).astype(bf16),
            "bout": (b_out if g == 0 else np.zeros_like(b_out))[None].astype(bf16),
            "cosT": cosT,
            "sinT": sinT,
        })
    res = run_bass_kernel_spmd(nc, in_maps, list(range(8)))
    _NC_CACHE["last_results"] = res
    parts = [r["out"] for r in res.results]
    return np.stack([parts[2 * b] + parts[2 * b + 1] for b in range(4)]).astype(np.float32)



# revision 29
# speedup vs baseline: 1.0553x; 1.0553x over previous
"""Trainium2 Bass kernel for multi-head attention (B=4, L=2048, D=1024, H=16).

Sharding: 8 cores = 4 batches x 2 head-groups (8 heads each).

Design notes (cost-model-driven):
- Scores matmuls run in fp8-e4m3 with DoubleRow perf mode.  Each head's
  64 features sit on a 64-partition window (2 heads per 128 partitions,
  bases 0/64); the DR slot pair is (real features, shared zero slice)
  addressed by a step-sliced AP, so one DR matmul contracts the whole
  head at 0.5 cycles/row -- half the bf16 cost.
- PV runs in bf16 with q on the OUTPUT PARTITIONS: out [128 q, 65] with
  exp as the stationary operand, so the per-instruction moving size is
  65 (64 v-features + a ones column = softmax denominator).  This halves
  PV cost vs. the [65, 512] orientation.
- RoPE rotate-half: front-critical tiles get it as a SECOND projection
  with row-swapped weights (wqkT slices 8/9, PE is idle early); steady
  state uses DVE stream_shuffle copies (partition-offset views).  The
  sign of the rotated term is baked into the sin table.
- Attention out [q, hd] is normalized by a per-partition scalar
  (1/denominator), then transposed feat-major via identity matmuls so
  the output projection can contract features on partitions.
- exp on ACT from 3-bank/2-bank PSUM groups; ACT (~255us busy) is the
  bottleneck engine and everything else is scheduled to hide under it.
- Projections / ropes / out-projection stream into the bursts as PE
  filler via a named pending-closure queue; require() guarantees
  writers are always emitted before readers, V projection is emitted
  inline just-in-time, and dummy matmuls keep the PE p-state warm when
  filler runs out.
"""
import sys

sys.path.insert(0, "/opt/trn_rl_repo")
import numpy as np
import concourse.bass as bass
import concourse.bacc as bacc
import concourse.mybir as mybir
from concourse.tile import TileContext
from concourse.bass_utils import run_bass_kernel_spmd

L = 2048          # sequence length
D = 1024          # model dim
HD = 64           # head dim
KT = L // 128     # 16 k position tiles
QC = 4            # q chunks of 512
DT = mybir.dt.bfloat16
F32 = mybir.dt.float32
F8 = mybir.dt.float8e4
SCALE = HD ** -0.5
AF = mybir.ActivationFunctionType
DR = mybir.MatmulPerfMode.DoubleRow

N_WARM = 75       # PE p-state warmup matmuls (until first wave matmul)


def build_nc():
    nc = bacc.Bacc("TRN2", target_bir_lowering=False, debug=False, num_devices=8)
    xT = nc.dram_tensor("xT", [D, L], DT, kind="ExternalInput")
    # slices 0-7: q/k projection weights; 8/9: row-swapped fc0/fc4
    wqkT = nc.dram_tensor("wqkT", [10, 128, 8, 128], DT, kind="ExternalInput")
    wvT = nc.dram_tensor("wvT", [D, 512], DT, kind="ExternalInput")
    bqk = nc.dram_tensor("bqk", [128, 10], F32, kind="ExternalInput")
    bv = nc.dram_tensor("bv", [1, 512], DT, kind="ExternalInput")
    woT = nc.dram_tensor("woT", [512, D], DT, kind="ExternalInput")
    cosT = nc.dram_tensor("cosT", [128, L], DT, kind="ExternalInput")
    sinT = nc.dram_tensor("sinT", [128, L], DT, kind="ExternalInput")
    ident = nc.dram_tensor("ident", [128, 128], DT, kind="ExternalInput")
    zeros8 = nc.dram_tensor("zeros8", [128, L], F8, kind="ExternalInput")
    out = nc.dram_tensor("out", [L, D], F32, kind="ExternalOutput")

    with TileContext(nc) as tc:
        with (
            tc.tile_pool(name="const", bufs=1) as cp,
            tc.tile_pool(name="wstream", bufs=1) as wsp,
            tc.tile_pool(name="rb", bufs=6) as rbp,
            tc.tile_pool(name="rtmp", bufs=2) as rtp,
            tc.tile_pool(name="exps", bufs=2) as ep,
            tc.tile_pool(name="apair", bufs=2) as app,
            tc.tile_pool(name="ctile", bufs=2) as ctp,
            tc.tile_pool(name="osb", bufs=3) as osp,
            tc.tile_pool(name="small", bufs=2) as sp,
            tc.tile_pool(name="psum", bufs=1, space="PSUM") as pp,
        ):
            dma = nc.default_dma_engine     # SP / HWDGE
            dma2 = nc.gpsimd                # Pool / SWDGE

            scratch = cp.tile([1, 640], DT)
            nc.vector.memset(scratch[:], 0.0)

            wqk_tiles = {}

            def fetch_wqk(fc, eng=dma):
                t = wsp.tile([128, 8, 128], DT, tag=f"wqk{fc}", name=f"wqk{fc}")
                eng.dma_start(out=t[:], in_=wqkT[fc])
                wqk_tiles[fc] = t

            # qkT8: data slices 0-7 (fc order), shared zero slice at 8.
            qkT8 = cp.tile([128, 9, L], F8)
            xT_sb = cp.tile([128, 8, L], DT)

            # DMA order: xT + the four wave weight slices first.
            dma.dma_start(out=xT_sb[:, 0, :], in_=xT[0:128, :])
            dma2.dma_start(out=xT_sb[:, 1, :], in_=xT[128:256, :])
            fetch_wqk(0, dma)
            fetch_wqk(8, dma)
            fetch_wqk(4, dma2)
            fetch_wqk(9, dma2)
            for c in range(2, 8):
                (dma, dma2)[c % 2].dma_start(out=xT_sb[:, c, :],
                                             in_=xT[c * 128:(c + 1) * 128, :])
            bqk_sb = cp.tile([128, 10], F32)
            dma2.dma_start(out=bqk_sb[:], in_=bqk[:])
            dma.dma_start(out=qkT8[:, 8, :], in_=zeros8[:])
            cos_sb = cp.tile([128, L], DT)
            dma.dma_start(out=cos_sb[:], in_=cosT[:])
            sin_sb = cp.tile([128, L], DT)
            dma.dma_start(out=sin_sb[:], in_=sinT[:])
            ident_sb = cp.tile([128, 128], DT)
            dma2.dma_start(out=ident_sb[:], in_=ident[:])
            bv_sb = cp.tile([1, 512], DT)
            dma2.dma_start(out=bv_sb[:], in_=bv[:])
            bv_bc = cp.tile([128, 512], DT)
            nc.gpsimd.partition_broadcast(bv_bc[:], bv_sb[:])
            wvT_sb = cp.tile([128, 8, 512], DT)
            woT_sb = cp.tile([128, 4, D], DT)

            V_sb = cp.tile([128, KT, 8 * (HD + 1)], DT)
            v4 = V_sb[:].rearrange("p k (h c) -> p k h c", c=HD + 1)
            nc.vector.memset(v4[:, :, :, HD:HD + 1], 1.0)

            # ---- PE warmup (p-state ramp) until first wave matmul ----
            warm = pp.tile([128, 128], F32, tag="ot", bufs=2, name="warm")
            for _ in range(N_WARM):
                nc.tensor.matmul(warm[:], lhsT=scratch[0:1, 0:128],
                                 rhs=scratch[0:1, 128:256], start=True, stop=True)

            def zero_bank(ap):
                nc.tensor.matmul(ap, lhsT=scratch[0:1, 0:128],
                                 rhs=scratch[0:1, 128:128 + ap.shape[-1]],
                                 start=True, stop=False, skip_group_check=True)

            def dummy_fill(n):
                dm = pp.tile([128, 256], F32, tag="ot", bufs=2, name="dm")
                for _ in range(n):
                    nc.tensor.matmul(dm[:], lhsT=scratch[0:1, 0:128],
                                     rhs=scratch[0:1, 128:384],
                                     start=True, stop=True)

            rb_tiles = {}

            def rb_tile(fc):
                if fc not in rb_tiles:
                    rb_tiles[fc] = rbp.tile([128, L], DT, tag="rb", name=f"rb{fc}")
                return rb_tiles[fc]

            def rope_dve(fc, nt, rot):
                """qkT8[fc] <- rb*cos + rot*sin for positions nt*512.."""
                rb = rb_tiles[fc]
                ntr = slice(nt * 512, (nt + 1) * 512)
                tmp = rtp.tile([128, 512], DT, tag="rtmp", name=f"rm{fc}_{nt}")
                nc.vector.tensor_mul(tmp[:], rb[:, ntr], cos_sb[:, ntr])
                nc.vector.tensor_mul(rot, rot, sin_sb[:, ntr])
                nc.vector.tensor_add(qkT8[:, fc, ntr], tmp[:], rot)

            def rope_shuffle(fc, nt):
                """rotate-half via DVE stream_shuffle (steady-state path)."""
                rb = rb_tiles[fc]
                ntr = slice(nt * 512, (nt + 1) * 512)
                rot = rtp.tile([128, 512], DT, tag="rot", name=f"rt{fc}_{nt}")
                idm = list(range(32))
                for h2 in range(2):
                    p = 64 * h2
                    nc.vector.stream_shuffle(rot[p:p + 32, :],
                                             rb[p + 32:p + 64, ntr], idm)
                    nc.vector.stream_shuffle(rot[p + 32:p + 64, :],
                                             rb[p:p + 32, ntr], idm)
                rope_dve(fc, nt, rot[:])

            def bias_to(dst, acc, col):
                nc.vector.tensor_scalar_add(dst, acc, bqk_sb[:, col:col + 1])

            def qk_proj_now(fc, nt):
                ot = pp.tile([128, 512], F32, tag="ot", bufs=2, name=f"qp{fc}_{nt}")
                for kc in range(8):
                    nc.tensor.matmul(ot[:], lhsT=wqk_tiles[fc][:, kc, :],
                                     rhs=xT_sb[:, kc, nt * 512:(nt + 1) * 512],
                                     start=(kc == 0), stop=(kc == 7))
                bias_to(rb_tile(fc)[:, nt * 512:(nt + 1) * 512], ot[:], fc)

            def rope_rotproj(fc, nt, rot_acc=None):
                """rotate-half via a projection with row-swapped weights."""
                if rot_acc is None:
                    ra = pp.tile([128, 512], F32, tag="ot", bufs=2,
                                 name=f"qr{fc}_{nt}")
                    wr = wqk_tiles[8 if fc == 0 else 9]
                    for kc in range(8):
                        nc.tensor.matmul(
                            ra[:], lhsT=wr[:, kc, :],
                            rhs=xT_sb[:, kc, nt * 512:(nt + 1) * 512],
                            start=(kc == 0), stop=(kc == 7))
                    rot_acc = ra[:]
                rot = rtp.tile([128, 512], DT, tag="rot", name=f"rr{fc}_{nt}")
                bias_to(rot[:], rot_acc, 8 if fc == 0 else 9)
                rope_dve(fc, nt, rot[:])

            # ---- wave: kc-outer, 6 accumulators: q/k nt0 of heads 0-1
            #      (+ their rotated projections) and k nt1-2 ----
            spA = pp.tile([128, 3, 512], F32, tag="sA", name="waveA")
            spB = pp.tile([128, 2, 512], F32, tag="sB", name="waveB")
            ot_w = pp.tile([128, 512], F32, tag="ot", bufs=2, name="waveO")
            wave = [(0, 0, spA[:, 0, :]), (8, 0, spA[:, 1, :]),
                    (4, 0, spA[:, 2, :]), (9, 0, spB[:, 0, :]),
                    (4, 1, spB[:, 1, :]), (4, 2, ot_w[:])]
            for kc in range(8):
                for fc, nt, acc in wave:
                    nc.tensor.matmul(acc, lhsT=wqk_tiles[fc][:, kc, :],
                                     rhs=xT_sb[:, kc, nt * 512:(nt + 1) * 512],
                                     start=(kc == 0), stop=(kc == 7))
            accs = {(fc, nt): acc for fc, nt, acc in wave}
            # q/k nt0 units via the rotated projections
            bias_to(rb_tile(0)[:, 0:512], accs[(0, 0)], 0)
            rope_rotproj(0, 0, rot_acc=accs[(8, 0)])
            bias_to(rb_tile(4)[:, 0:512], accs[(4, 0)], 4)
            rope_rotproj(4, 0, rot_acc=accs[(9, 0)])
            bias_to(rb_tile(4)[:, 512:1024], accs[(4, 1)], 4)
            bias_to(rb_tile(4)[:, 1024:1536], accs[(4, 2)], 4)
            # k nt1-2 rot tiles (PE is free while early bursts are ACT-bound)
            rope_rotproj(4, 1)
            rope_rotproj(4, 2)

            # ---- named filler queue ----
            pending = []     # list of (name, closure)
            done = {"r0_0", "r4_0", "r4_1", "r4_2"}

            def run_next():
                name, fn = pending.pop(0)
                fn()
                done.add(name)

            def drain(n):
                for _ in range(min(n, len(pending))):
                    run_next()

            def require(name):
                if name in done:
                    return
                assert any(n == name for n, _ in pending), f"missing {name}"
                while name not in done:
                    run_next()

            def qk_proj(fc, nt):
                return (f"qp{fc}_{nt}", lambda: qk_proj_now(fc, nt))

            def rope_f(fc, nt):
                return (f"r{fc}_{nt}", lambda: rope_shuffle(fc, nt))

            def rope_rp(fc, nt):
                return (f"r{fc}_{nt}", lambda: rope_rotproj(fc, nt))

            def fetch_f(fc):
                return (f"fw{fc}", lambda: fetch_wqk(fc))

            def fetch_wv():
                for c in range(8):
                    (dma, dma2)[c % 2].dma_start(
                        out=wvT_sb[:, c, :], in_=wvT[c * 128:(c + 1) * 128, :])

            def fetch_wo():
                for c in range(4):
                    dma2.dma_start(out=woT_sb[:, c, :],
                                   in_=woT[c * 128:(c + 1) * 128, :])

            def kchain(ch2):
                fk, fq = 4 + ch2, ch2
                items = [fetch_f(fk)]
                for nt in range(4):
                    items += [qk_proj(fk, nt), rope_f(fk, nt)]
                items += [fetch_f(fq)]
                for nt in range(4):
                    items += [qk_proj(fq, nt), rope_f(fq, nt)]
                return items

            q0chain = []
            for nt in range(1, 4):
                q0chain += [qk_proj(0, nt), rope_f(0, nt)]

            pending.extend(
                [qk_proj(4, 3), rope_rp(4, 3), ("fwv", fetch_wv)] +
                q0chain + kchain(1) + [("fwo", fetch_wo)] +
                kchain(2) + kchain(3))

            # ---- V projection: emitted inline, just in time ----
            v_done = set()

            def v_need(lt, hp):
                if (lt, hp) in v_done:
                    return
                require("fwv")
                v_done.add((lt, hp))
                ot = pp.tile([128, 128], F32, tag="ot", bufs=2, name=f"vp{lt}_{hp}")
                for kc in range(8):
                    nc.tensor.matmul(
                        ot[:],
                        lhsT=xT_sb[:, kc, lt * 128:(lt + 1) * 128],
                        rhs=wvT_sb[:, kc, hp * 128:(hp + 1) * 128],
                        start=(kc == 0), stop=(kc == 7))
                nc.vector.tensor_add(
                    v4[:, lt, 2 * hp:2 * hp + 2, 0:HD],
                    ot[:].rearrange("p (h c) -> p h c", c=HD),
                    bv_bc[:, hp * 128:(hp + 1) * 128]
                    .rearrange("p (h c) -> p h c", c=HD))

            # ---- attention bursts ----
            GROUPS = [(0, 3, "sA"), (3, 2, "sB"), (5, 3, "sA"),
                      (8, 2, "sB"), (10, 3, "sA"), (13, 3, "sA")]
            KROPE_NT = {0: 0, 3: 1, 5: 1, 8: 2, 10: 3, 13: 3}

            def out_proj(cT, qc, dt_, mq):
                def emit():
                    require("fwo")
                    ops = pp.tile([128, 512], F32, tag="ot", bufs=2,
                                  name=f"op{qc}{dt_}{mq}")
                    for cc in range(4):
                        nc.tensor.matmul(ops[:], lhsT=cT[:, cc, qc, mq, :],
                                         rhs=woT_sb[:, cc, dt_ * 512:(dt_ + 1) * 512],
                                         start=(cc == 0), stop=(cc == 3))
                    o = osp.tile([128, 512], F32, tag="osb", name=f"os{qc}{dt_}{mq}")
                    nc.vector.tensor_copy(o[:], ops[:])
                    dma.dma_start(
                        out=out[qc * 512 + mq * 128: qc * 512 + (mq + 1) * 128,
                                dt_ * 512:(dt_ + 1) * 512],
                        in_=o[:])
                return (f"op{qc}_{dt_}_{mq}", emit)

            cT = ctp.tile([128, 4, QC, 4, 128], DT, tag="cT", bufs=1, name="cT")
            apair_box = [None]

            def burst(h, qc):
                ch2 = h // 2
                prow = slice(64 * (h % 2), 64 * (h % 2) + 64)
                eq, ek = ch2, 4 + ch2
                require(f"r{ch2}_{qc}")
                exp_t = ep.tile([128, KT, 512], DT, tag="exp", name=f"ex{qc}{h}")
                pv = pp.tile([128, 512], F32, tag="pvx", name=f"pv{qc}{h}")
                zero_bank(pv[:])
                pvv = pv[:, 0:320].rearrange("p (q c) -> p q c", c=80)
                if h % 2 == 0:
                    apair_box[0] = app.tile([128, 4, 2, HD], DT, tag="ap",
                                            name=f"ap{qc}{h}")
                apair = apair_box[0]

                def pv_group(kt0, n):
                    for lt in range(kt0, kt0 + n):
                        v_need(lt, ch2)
                    for i in range(n):
                        kt = kt0 + i
                        for qt in range(4):
                            nc.tensor.matmul(
                                pvv[:, qt, 0:65],
                                lhsT=exp_t[:, kt, qt * 128:(qt + 1) * 128],
                                rhs=V_sb[:, kt, h * 65:(h + 1) * 65],
                                start=False, stop=(kt == KT - 1),
                                skip_group_check=True)

                prev = None
                for gi, (kt0, n, tag) in enumerate(GROUPS):
                    require(f"r{4 + ch2}_{KROPE_NT[kt0]}")
                    sg = pp.tile([128, n, 512], F32, tag=tag,
                                 name=f"sg{qc}{h}{kt0}")
                    for i in range(n):
                        kt = kt0 + i
                        nc.tensor.matmul(
                            sg[:, i, :],
                            lhsT=qkT8[prow, ek:9:8 - ek, kt * 128:(kt + 1) * 128],
                            rhs=qkT8[prow, eq:9:8 - eq, qc * 512:(qc + 1) * 512],
                            start=True, stop=True, perf_mode=DR)
                    nc.scalar.activation(
                        exp_t[:, kt0:kt0 + n, :].rearrange("p a b -> p (a b)"),
                        sg[:].rearrange("p a b -> p (a b)"), AF.Exp, scale=SCALE)
                    if gi in (1, 3, 4):
                        if pending:
                            drain(1)
                        else:
                            dummy_fill(3)
                    if prev is not None:
                        pv_group(prev[0], prev[1])
                    prev = (kt0, n)
                pv_group(prev[0], prev[1])

                r = sp.tile([128, 4], F32, tag="rsb", name=f"r{qc}{h}")
                nc.vector.reciprocal(
                    r[:], pvv[:, :, 64:65].rearrange("p q c -> p (q c)"))
                for qt in range(4):
                    nc.vector.tensor_scalar_mul(
                        apair[:, qt, h % 2, :],
                        pvv[:, qt, 0:64], r[:, qt:qt + 1])

                if h % 2 == 1:
                    xp = pp.tile([128, 4, 128], F32, tag="pvx",
                                 name=f"xp{qc}{ch2}")
                    zero_bank(xp[:].rearrange("p a b -> p (a b)"))
                    for qt in range(4):
                        nc.tensor.matmul(
                            xp[:, qt, :],
                            lhsT=apair[:, qt, :, :].rearrange("p a b -> p (a b)"),
                            rhs=ident_sb[:], start=False, stop=True,
                            skip_group_check=True)
                    nc.vector.tensor_copy(cT[:, ch2, qc, :, :], xp[:])
                    if pending:
                        drain(1)

            for hp in range(4):
                for qc in range(QC):
                    burst(2 * hp, qc)
                    burst(2 * hp + 1, qc)
                    if hp == 3:
                        pending.extend(out_proj(cT, qc, dt_, mq)
                                       for dt_ in range(2) for mq in range(4))
            while pending:
                run_next()
    nc.compile()
    return nc


def _rope_tables_np():
    import ml_dtypes
    bf16 = ml_dtypes.bfloat16
    inv_freq = 1.0 / (10000.0 ** (np.arange(0, HD, 2, dtype=np.float32) / HD))
    t = np.arange(L, dtype=np.float32)
    freqs = np.outer(t, inv_freq).astype(np.float32)       # [L, 32]
    cos_h = np.cos(freqs).T                                # [32, L]
    sin_h = np.sin(freqs).T
    cosT = np.concatenate([cos_h, cos_h], 0)               # [64, L]
    sinT = np.concatenate([-sin_h, sin_h], 0)              # sign baked for rot trick
    return (np.tile(cosT, (2, 1)).astype(bf16),
            np.tile(sinT, (2, 1)).astype(bf16))            # [128, L]


_NC_CACHE = {}


def kernel(x, w_qkv, b_qkv, w_out, b_out):
    import ml_dtypes
    bf16 = ml_dtypes.bfloat16
    f8 = ml_dtypes.float8_e4m3
    if "nc" not in _NC_CACHE:
        _NC_CACHE["nc"] = build_nc()
    nc = _NC_CACHE["nc"]

    cosT, sinT = _rope_tables_np()
    ident = np.eye(128, dtype=np.float32).astype(bf16)
    zeros8 = np.zeros((128, L), dtype=f8)
    p = np.arange(128)
    swap = (p // 64) * 64 + ((p % 64) + 32) % 64
    in_maps = []
    for core in range(8):
        b, g = divmod(core, 2)
        s = slice(512 * g, 512 * (g + 1))
        wqk = np.concatenate([w_qkv[0:D][s], w_qkv[D:2 * D][s]], 0)  # [1024, 1024]
        bqk_v = np.concatenate([b_qkv[0:D][s], b_qkv[D:2 * D][s]])
        # [8, 128, 8, 128] = [fc, dpart, kc, m]
        wqk_t = wqk.T.reshape(8, 128, 8, 128).transpose(2, 1, 0, 3)
        # swapped-row variants of fc0 / fc4 (rotate-half projections);
        # m indexes output features, so permute the last axis
        wswap = np.stack([wqk_t[0][:, :, swap], wqk_t[4][:, :, swap]])
        bqk_cols = np.concatenate(
            [bqk_v.reshape(8, 128).T,
             bqk_v.reshape(8, 128)[[0, 4]].T[swap]], axis=1)   # [128, 10]
        in_maps.append({
            "xT": np.ascontiguousarray(x[b].T).astype(bf16),
            "wqkT": np.ascontiguousarray(
                np.concatenate([wqk_t, wswap], 0)).astype(bf16),
            "wvT": np.ascontiguousarray(w_qkv[2 * D:3 * D][s].T).astype(bf16),
            "bqk": np.ascontiguousarray(bqk_cols).astype(np.float32),
            "bv": b_qkv[2 * D:3 * D][s][None].astype(bf16),
            "woT": np.ascontiguousarray(w_out[:, s].T).astype(bf16),
            "cosT": cosT,
            "sinT": sinT,
            "ident": ident,
            "zeros8": zeros8,
        })
    res = run_bass_kernel_spmd(nc, in_maps, list(range(8)))
    _NC_CACHE["last_results"] = res
    parts = [r["out"] for r in res.results]
    full = np.stack([parts[2 * b] + parts[2 * b + 1] for b in range(4)])
    return (full + b_out[None, None, :]).astype(np.float32)


# revision 30
# speedup vs baseline: 1.0711x; 1.0150x over previous
"""Trainium2 Bass kernel for multi-head attention (B=4, L=2048, D=1024, H=16).

Sharding: 8 cores = 4 batches x 2 head-groups (8 heads each).

Design notes (cost-model-driven):
- Scores matmuls run in fp8-e4m3 with DoubleRow perf mode.  Each head's
  64 features sit on a 64-partition window (2 heads per 128 partitions,
  bases 0/64); the DR slot pair is (real features, shared zero slice)
  addressed by a step-sliced AP, so one DR matmul contracts the whole
  head at 0.5 cycles/row -- half the bf16 cost.
- PV runs in bf16 with q on the OUTPUT PARTITIONS: out [128 q, 65] with
  exp as the stationary operand, so the per-instruction moving size is
  65 (64 v-features + a ones column = softmax denominator).  This halves
  PV cost vs. the [65, 512] orientation.
- RoPE rotate-half: front-critical tiles get it as a SECOND projection
  with row-swapped weights (wqkT slices 8/9, PE is idle early); steady
  state uses DVE stream_shuffle copies (partition-offset views).  The
  sign of the rotated term is baked into the sin table.
- Attention out [q, hd] is normalized by a per-partition scalar
  (1/denominator), then transposed feat-major via identity matmuls so
  the output projection can contract features on partitions.
- exp on ACT from 3-bank/2-bank PSUM groups; ACT (~255us busy) is the
  bottleneck engine and everything else is scheduled to hide under it.
- Projections / ropes / out-projection stream into the bursts as PE
  filler via a named pending-closure queue; require() guarantees
  writers are always emitted before readers, V projection is emitted
  inline just-in-time, and dummy matmuls keep the PE p-state warm when
  filler runs out.
"""
import sys

sys.path.insert(0, "/opt/trn_rl_repo")
import numpy as np
import concourse.bass as bass
import concourse.bacc as bacc
import concourse.mybir as mybir
from concourse.tile import TileContext
from concourse.bass_utils import run_bass_kernel_spmd

L = 2048          # sequence length
D = 1024          # model dim
HD = 64           # head dim
KT = L // 128     # 16 k position tiles
QC = 4            # q chunks of 512
DT = mybir.dt.bfloat16
F32 = mybir.dt.float32
F8 = mybir.dt.float8e4
SCALE = HD ** -0.5
AF = mybir.ActivationFunctionType
DR = mybir.MatmulPerfMode.DoubleRow

N_WARM = 75       # PE p-state warmup matmuls (until first wave matmul)


def build_nc():
    nc = bacc.Bacc("TRN2", target_bir_lowering=False, debug=False, num_devices=8)
    xT = nc.dram_tensor("xT", [D, L], DT, kind="ExternalInput")
    # slices 0-7: q/k projection weights; 8/9: row-swapped fc0/fc4
    wqkT = nc.dram_tensor("wqkT", [10, 128, 8, 128], DT, kind="ExternalInput")
    wvT = nc.dram_tensor("wvT", [D, 512], DT, kind="ExternalInput")
    bqk = nc.dram_tensor("bqk", [128, 10], F32, kind="ExternalInput")
    bv = nc.dram_tensor("bv", [1, 512], DT, kind="ExternalInput")
    woT = nc.dram_tensor("woT", [512, D], DT, kind="ExternalInput")
    cosT = nc.dram_tensor("cosT", [128, L], DT, kind="ExternalInput")
    sinT = nc.dram_tensor("sinT", [128, L], DT, kind="ExternalInput")
    ident = nc.dram_tensor("ident", [128, 128], DT, kind="ExternalInput")
    zeros8 = nc.dram_tensor("zeros8", [128, L], F8, kind="ExternalInput")
    out = nc.dram_tensor("out", [L, D], F32, kind="ExternalOutput")

    with TileContext(nc) as tc:
        with (
            tc.tile_pool(name="const", bufs=1) as cp,
            tc.tile_pool(name="wstream", bufs=1) as wsp,
            tc.tile_pool(name="rb", bufs=6) as rbp,
            tc.tile_pool(name="rtmp", bufs=2) as rtp,
            tc.tile_pool(name="exps", bufs=2) as ep,
            tc.tile_pool(name="apair", bufs=2) as app,
            tc.tile_pool(name="ctile", bufs=2) as ctp,
            tc.tile_pool(name="osb", bufs=3) as osp,
            tc.tile_pool(name="small", bufs=2) as sp,
            tc.tile_pool(name="psum", bufs=1, space="PSUM") as pp,
        ):
            dma = nc.default_dma_engine     # SP / HWDGE
            dma2 = nc.gpsimd                # Pool / SWDGE

            scratch = cp.tile([1, 640], DT)
            nc.vector.memset(scratch[:], 0.0)

            wqk_tiles = {}

            def fetch_wqk(fc, eng=dma):
                t = wsp.tile([128, 8, 128], DT, tag=f"wqk{fc}", name=f"wqk{fc}")
                eng.dma_start(out=t[:], in_=wqkT[fc])
                wqk_tiles[fc] = t

            # qkT8: data slices 0-7 (fc order), shared zero slice at 8.
            qkT8 = cp.tile([128, 9, L], F8)
            xT_sb = cp.tile([128, 8, L], DT)

            # DMA order: xT + the four wave weight slices first.
            dma.dma_start(out=xT_sb[:, 0, :], in_=xT[0:128, :])
            dma2.dma_start(out=xT_sb[:, 1, :], in_=xT[128:256, :])
            fetch_wqk(0, dma)
            fetch_wqk(8, dma)
            fetch_wqk(4, dma2)
            fetch_wqk(9, dma2)
            for c in range(2, 8):
                (dma, dma2)[c % 2].dma_start(out=xT_sb[:, c, :],
                                             in_=xT[c * 128:(c + 1) * 128, :])
            bqk_sb = cp.tile([128, 10], F32)
            dma2.dma_start(out=bqk_sb[:], in_=bqk[:])
            dma.dma_start(out=qkT8[:, 8, :], in_=zeros8[:])
            cos_sb = cp.tile([128, L], DT)
            dma.dma_start(out=cos_sb[:], in_=cosT[:])
            sin_sb = cp.tile([128, L], DT)
            dma.dma_start(out=sin_sb[:], in_=sinT[:])
            ident_sb = cp.tile([128, 128], DT)
            dma2.dma_start(out=ident_sb[:], in_=ident[:])
            bv_sb = cp.tile([1, 512], DT)
            dma2.dma_start(out=bv_sb[:], in_=bv[:])
            bv_bc = cp.tile([128, 512], DT)
            nc.gpsimd.partition_broadcast(bv_bc[:], bv_sb[:])
            wvT_sb = cp.tile([128, 8, 512], DT)
            woT_sb = cp.tile([128, 4, D], DT)

            V_sb = cp.tile([128, KT, 8 * (HD + 1)], DT)
            v4 = V_sb[:].rearrange("p k (h c) -> p k h c", c=HD + 1)
            nc.vector.memset(v4[:, :, :, HD:HD + 1], 1.0)

            # ---- PE warmup (p-state ramp) until first wave matmul ----
            warm = pp.tile([128, 128], F32, tag="ot", bufs=1, name="warm")
            for _ in range(N_WARM):
                nc.tensor.matmul(warm[:], lhsT=scratch[0:1, 0:128],
                                 rhs=scratch[0:1, 128:256], start=True, stop=True)

            def zero_bank(ap):
                nc.tensor.matmul(ap, lhsT=scratch[0:1, 0:128],
                                 rhs=scratch[0:1, 128:128 + ap.shape[-1]],
                                 start=True, stop=False, skip_group_check=True)

            def dummy_fill(n):
                dm = pp.tile([128, 256], F32, tag="ot", bufs=1, name="dm")
                for _ in range(n):
                    nc.tensor.matmul(dm[:], lhsT=scratch[0:1, 0:128],
                                     rhs=scratch[0:1, 128:384],
                                     start=True, stop=True)

            rb_tiles = {}

            def rb_tile(fc):
                if fc not in rb_tiles:
                    rb_tiles[fc] = rbp.tile([128, L], DT, tag="rb", name=f"rb{fc}")
                return rb_tiles[fc]

            def rope_dve(fc, nt, rot):
                """qkT8[fc] <- rb*cos + rot*sin for positions nt*512.."""
                rb = rb_tiles[fc]
                ntr = slice(nt * 512, (nt + 1) * 512)
                tmp = rtp.tile([128, 512], DT, tag="rtmp", name=f"rm{fc}_{nt}")
                nc.vector.tensor_mul(tmp[:], rb[:, ntr], cos_sb[:, ntr])
                nc.vector.tensor_mul(rot, rot, sin_sb[:, ntr])
                nc.vector.tensor_add(qkT8[:, fc, ntr], tmp[:], rot)

            def rope_shuffle(fc, nt):
                """rotate-half via DVE stream_shuffle (steady-state path)."""
                rb = rb_tiles[fc]
                ntr = slice(nt * 512, (nt + 1) * 512)
                rot = rtp.tile([128, 512], DT, tag="rot", name=f"rt{fc}_{nt}")
                idm = list(range(32))
                for h2 in range(2):
                    p = 64 * h2
                    nc.vector.stream_shuffle(rot[p:p + 32, :],
                                             rb[p + 32:p + 64, ntr], idm)
                    nc.vector.stream_shuffle(rot[p + 32:p + 64, :],
                                             rb[p:p + 32, ntr], idm)
                rope_dve(fc, nt, rot[:])

            def bias_to(dst, acc, col):
                nc.vector.tensor_scalar_add(dst, acc, bqk_sb[:, col:col + 1])

            def qk_proj_now(fc, nt):
                ot = pp.tile([128, 512], F32, tag="ot", bufs=1, name=f"qp{fc}_{nt}")
                for kc in range(8):
                    nc.tensor.matmul(ot[:], lhsT=wqk_tiles[fc][:, kc, :],
                                     rhs=xT_sb[:, kc, nt * 512:(nt + 1) * 512],
                                     start=(kc == 0), stop=(kc == 7))
                bias_to(rb_tile(fc)[:, nt * 512:(nt + 1) * 512], ot[:], fc)

            def rope_rotproj(fc, nt, rot_acc=None):
                """rotate-half via a projection with row-swapped weights."""
                if rot_acc is None:
                    ra = pp.tile([128, 512], F32, tag="ot", bufs=1,
                                 name=f"qr{fc}_{nt}")
                    wr = wqk_tiles[8 if fc == 0 else 9]
                    for kc in range(8):
                        nc.tensor.matmul(
                            ra[:], lhsT=wr[:, kc, :],
                            rhs=xT_sb[:, kc, nt * 512:(nt + 1) * 512],
                            start=(kc == 0), stop=(kc == 7))
                    rot_acc = ra[:]
                rot = rtp.tile([128, 512], DT, tag="rot", name=f"rr{fc}_{nt}")
                bias_to(rot[:], rot_acc, 8 if fc == 0 else 9)
                rope_dve(fc, nt, rot[:])

            # ---- wave: kc-outer, 6 accumulators: q/k nt0 of heads 0-1
            #      (+ their rotated projections) and k nt1-2 ----
            spA = pp.tile([128, 3, 512], F32, tag="sA", name="waveA")
            spB = pp.tile([128, 3, 512], F32, tag="sB", name="waveB")
            wave = [(0, 0, spA[:, 0, :]), (8, 0, spA[:, 1, :]),
                    (4, 0, spA[:, 2, :]), (9, 0, spB[:, 0, :]),
                    (4, 1, spB[:, 1, :]), (4, 2, spB[:, 2, :])]
            for kc in range(8):
                for fc, nt, acc in wave:
                    nc.tensor.matmul(acc, lhsT=wqk_tiles[fc][:, kc, :],
                                     rhs=xT_sb[:, kc, nt * 512:(nt + 1) * 512],
                                     start=(kc == 0), stop=(kc == 7))
            accs = {(fc, nt): acc for fc, nt, acc in wave}
            # q/k nt0 units via the rotated projections
            bias_to(rb_tile(0)[:, 0:512], accs[(0, 0)], 0)
            rope_rotproj(0, 0, rot_acc=accs[(8, 0)])
            bias_to(rb_tile(4)[:, 0:512], accs[(4, 0)], 4)
            rope_rotproj(4, 0, rot_acc=accs[(9, 0)])
            bias_to(rb_tile(4)[:, 512:1024], accs[(4, 1)], 4)
            bias_to(rb_tile(4)[:, 1024:1536], accs[(4, 2)], 4)
            # k nt1-2 rot tiles (PE is free while early bursts are ACT-bound)
            rope_rotproj(4, 1)
            rope_rotproj(4, 2)

            # ---- named filler queue ----
            pending = []     # list of (name, closure)
            done = {"r0_0", "r4_0", "r4_1", "r4_2"}

            def run_next():
                name, fn = pending.pop(0)
                fn()
                done.add(name)

            def drain(n):
                for _ in range(min(n, len(pending))):
                    run_next()

            def require(name):
                if name in done:
                    return
                assert any(n == name for n, _ in pending), f"missing {name}"
                while name not in done:
                    run_next()

            def qk_proj(fc, nt):
                return (f"qp{fc}_{nt}", lambda: qk_proj_now(fc, nt))

            def rope_f(fc, nt):
                return (f"r{fc}_{nt}", lambda: rope_shuffle(fc, nt))

            def rope_rp(fc, nt):
                return (f"r{fc}_{nt}", lambda: rope_rotproj(fc, nt))

            def fetch_f(fc):
                return (f"fw{fc}", lambda: fetch_wqk(fc))

            def fetch_wv():
                for c in range(8):
                    (dma, dma2)[c % 2].dma_start(
                        out=wvT_sb[:, c, :], in_=wvT[c * 128:(c + 1) * 128, :])

            def fetch_wo():
                for c in range(4):
                    dma2.dma_start(out=woT_sb[:, c, :],
                                   in_=woT[c * 128:(c + 1) * 128, :])

            def kchain(ch2):
                fk, fq = 4 + ch2, ch2
                items = [fetch_f(fk)]
                for nt in range(4):
                    items += [qk_proj(fk, nt), rope_f(fk, nt)]
                items += [fetch_f(fq)]
                for nt in range(4):
                    items += [qk_proj(fq, nt), rope_f(fq, nt)]
                return items

            q0chain = []
            for nt in range(1, 4):
                q0chain += [qk_proj(0, nt), rope_f(0, nt)]

            pending.extend(
                [qk_proj(4, 3), rope_rp(4, 3), ("fwv", fetch_wv)] +
                q0chain + kchain(1) + [("fwo", fetch_wo)] +
                kchain(2) + kchain(3))

            # ---- V projection: emitted inline, just in time ----
            v_done = set()

            def v_need(lt, hp):
                if (lt, hp) in v_done:
                    return
                require("fwv")
                v_done.add((lt, hp))
                ot = pp.tile([128, 128], F32, tag="ot", bufs=1, name=f"vp{lt}_{hp}")
                for kc in range(8):
                    nc.tensor.matmul(
                        ot[:],
                        lhsT=xT_sb[:, kc, lt * 128:(lt + 1) * 128],
                        rhs=wvT_sb[:, kc, hp * 128:(hp + 1) * 128],
                        start=(kc == 0), stop=(kc == 7))
                nc.vector.tensor_add(
                    v4[:, lt, 2 * hp:2 * hp + 2, 0:HD],
                    ot[:].rearrange("p (h c) -> p h c", c=HD),
                    bv_bc[:, hp * 128:(hp + 1) * 128]
                    .rearrange("p (h c) -> p h c", c=HD))

            # ---- attention bursts ----
            GROUPS = [(0, 3, "sA"), (3, 3, "sB"), (6, 3, "sA"),
                      (9, 3, "sB"), (12, 2, "sA"), (14, 2, "sB")]
            KROPE_NT = {0: 0, 3: 1, 6: 2, 9: 2, 12: 3, 14: 3}

            def out_proj(cT, qc, dt_, mq):
                def emit():
                    require("fwo")
                    ops = pp.tile([128, 512], F32, tag="ot", bufs=1,
                                  name=f"op{qc}{dt_}{mq}")
                    for cc in range(4):
                        nc.tensor.matmul(ops[:], lhsT=cT[:, cc, qc, mq, :],
                                         rhs=woT_sb[:, cc, dt_ * 512:(dt_ + 1) * 512],
                                         start=(cc == 0), stop=(cc == 3))
                    o = osp.tile([128, 512], F32, tag="osb", name=f"os{qc}{dt_}{mq}")
                    nc.vector.tensor_copy(o[:], ops[:])
                    dma.dma_start(
                        out=out[qc * 512 + mq * 128: qc * 512 + (mq + 1) * 128,
                                dt_ * 512:(dt_ + 1) * 512],
                        in_=o[:])
                return (f"op{qc}_{dt_}_{mq}", emit)

            cT = ctp.tile([128, 4, QC, 4, 128], DT, tag="cT", bufs=1, name="cT")
            apair_box = [None]

            def burst(h, qc):
                ch2 = h // 2
                prow = slice(64 * (h % 2), 64 * (h % 2) + 64)
                eq, ek = ch2, 4 + ch2
                require(f"r{ch2}_{qc}")
                exp_t = ep.tile([128, KT, 512], DT, tag="exp", name=f"ex{qc}{h}")
                pv = pp.tile([128, 512], F32, tag="pvx", name=f"pv{qc}{h}")
                zero_bank(pv[:])
                pvv = pv[:, 0:320].rearrange("p (q c) -> p q c", c=80)
                if h % 2 == 0:
                    apair_box[0] = app.tile([128, 4, 2, HD], DT, tag="ap",
                                            name=f"ap{qc}{h}")
                apair = apair_box[0]

                def pv_group(kt0, n):
                    for lt in range(kt0, kt0 + n):
                        v_need(lt, ch2)
                    for i in range(n):
                        kt = kt0 + i
                        for qt in range(4):
                            nc.tensor.matmul(
                                pvv[:, qt, 0:65],
                                lhsT=exp_t[:, kt, qt * 128:(qt + 1) * 128],
                                rhs=V_sb[:, kt, h * 65:(h + 1) * 65],
                                start=False, stop=(kt == KT - 1),
                                skip_group_check=True)

                prev = None
                for gi, (kt0, n, tag) in enumerate(GROUPS):
                    require(f"r{4 + ch2}_{KROPE_NT[kt0]}")
                    sg = pp.tile([128, n, 512], F32, tag=tag,
                                 name=f"sg{qc}{h}{kt0}")
                    for i in range(n):
                        kt = kt0 + i
                        nc.tensor.matmul(
                            sg[:, i, :],
                            lhsT=qkT8[prow, ek:9:8 - ek, kt * 128:(kt + 1) * 128],
                            rhs=qkT8[prow, eq:9:8 - eq, qc * 512:(qc + 1) * 512],
                            start=True, stop=True, perf_mode=DR)
                    nc.scalar.activation(
                        exp_t[:, kt0:kt0 + n, :].rearrange("p a b -> p (a b)"),
                        sg[:].rearrange("p a b -> p (a b)"), AF.Exp, scale=SCALE)
                    if gi in (1, 3, 4):
                        if pending:
                            drain(1)
                        else:
                            dummy_fill(3)
                    if prev is not None:
                        pv_group(prev[0], prev[1])
                    prev = (kt0, n)
                pv_group(prev[0], prev[1])

                r = sp.tile([128, 4], F32, tag="rsb", name=f"r{qc}{h}")
                nc.vector.reciprocal(
                    r[:], pvv[:, :, 64:65].rearrange("p q c -> p (q c)"))
                for qt in range(4):
                    nc.vector.tensor_scalar_mul(
                        apair[:, qt, h % 2, :],
                        pvv[:, qt, 0:64], r[:, qt:qt + 1])

                if h % 2 == 1:
                    xp = pp.tile([128, 4, 128], F32, tag="pvx",
                                 name=f"xp{qc}{ch2}")
                    zero_bank(xp[:].rearrange("p a b -> p (a b)"))
                    for qt in range(4):
                        nc.tensor.matmul(
                            xp[:, qt, :],
                            lhsT=apair[:, qt, :, :].rearrange("p a b -> p (a b)"),
                            rhs=ident_sb[:], start=False, stop=True,
                            skip_group_check=True)
                    nc.vector.tensor_copy(cT[:, ch2, qc, :, :], xp[:])
                    if pending:
                        drain(1)

            for hp in range(4):
                for qc in range(QC):
                    burst(2 * hp, qc)
                    burst(2 * hp + 1, qc)
                    if hp == 3:
                        pending.extend(out_proj(cT, qc, dt_, mq)
                                       for dt_ in range(2) for mq in range(4))
            while pending:
                run_next()
    nc.compile()
    return nc


def _rope_tables_np():
    import ml_dtypes
    bf16 = ml_dtypes.bfloat16
    inv_freq = 1.0 / (10000.0 ** (np.arange(0, HD, 2, dtype=np.float32) / HD))
    t = np.arange(L, dtype=np.float32)
    freqs = np.outer(t, inv_freq).astype(np.float32)       # [L, 32]
    cos_h = np.cos(freqs).T                                # [32, L]
    sin_h = np.sin(freqs).T
    cosT = np.concatenate([cos_h, cos_h], 0)               # [64, L]
    sinT = np.concatenate([-sin_h, sin_h], 0)              # sign baked for rot trick
    return (np.tile(cosT, (2, 1)).astype(bf16),
            np.tile(sinT, (2, 1)).astype(bf16))            # [128, L]


_NC_CACHE = {}


def kernel(x, w_qkv, b_qkv, w_out, b_out):
    import ml_dtypes
    bf16 = ml_dtypes.bfloat16
    f8 = ml_dtypes.float8_e4m3
    if "nc" not in _NC_CACHE:
        _NC_CACHE["nc"] = build_nc()
    nc = _NC_CACHE["nc"]

    cosT, sinT = _rope_tables_np()
    ident = np.eye(128, dtype=np.float32).astype(bf16)
    zeros8 = np.zeros((128, L), dtype=f8)
    p = np.arange(128)
    swap = (p // 64) * 64 + ((p % 64) + 32) % 64
    in_maps = []
    for core in range(8):
        b, g = divmod(core, 2)
        s = slice(512 * g, 512 * (g + 1))
        wqk = np.concatenate([w_qkv[0:D][s], w_qkv[D:2 * D][s]], 0)  # [1024, 1024]
        bqk_v = np.concatenate([b_qkv[0:D][s], b_qkv[D:2 * D][s]])
        # [8, 128, 8, 128] = [fc, dpart, kc, m]
        wqk_t = wqk.T.reshape(8, 128, 8, 128).transpose(2, 1, 0, 3)
        # swapped-row variants of fc0 / fc4 (rotate-half projections);
        # m indexes output features, so permute the last axis
        wswap = np.stack([wqk_t[0][:, :, swap], wqk_t[4][:, :, swap]])
        bqk_cols = np.concatenate(
            [bqk_v.reshape(8, 128).T,
             bqk_v.reshape(8, 128)[[0, 4]].T[swap]], axis=1)   # [128, 10]
        in_maps.append({
            "xT": np.ascontiguousarray(x[b].T).astype(bf16),
            "wqkT": np.ascontiguousarray(
                np.concatenate([wqk_t, wswap], 0)).astype(bf16),
            "wvT": np.ascontiguousarray(w_qkv[2 * D:3 * D][s].T).astype(bf16),
            "bqk": np.ascontiguousarray(bqk_cols).astype(np.float32),
            "bv": b_qkv[2 * D:3 * D][s][None].astype(bf16),
            "woT": np.ascontiguousarray(w_out[:, s].T).astype(bf16),
            "cosT": cosT,
            "sinT": sinT,
            "ident": ident,
            "zeros8": zeros8,
        })
    res = run_bass_kernel_spmd(nc, in_maps, list(range(8)))
    _NC_CACHE["last_results"] = res
    parts = [r["out"] for r in res.results]
    full = np.stack([parts[2 * b] + parts[2 * b + 1] for b in range(4)])
    return (full + b_out[None, None, :]).astype(np.float32)


# revision 32
# speedup vs baseline: 1.0837x; 1.0118x over previous
"""Trainium2 Bass kernel for multi-head attention (B=4, L=2048, D=1024, H=16).

Sharding: 8 cores = 4 batches x 2 head-groups (8 heads each).

Design notes (cost-model-driven):
- Scores matmuls run in fp8-e4m3 with DoubleRow perf mode.  Each head's
  64 features sit on a 64-partition window (2 heads per 128 partitions,
  bases 0/64); the DR slot pair is (real features, shared zero slice)
  addressed by a step-sliced AP, so one DR matmul contracts the whole
  head at 0.5 cycles/row -- half the bf16 cost.
- PV runs in bf16 with q on the OUTPUT PARTITIONS: out [128 q, 65] with
  exp as the stationary operand, so the per-instruction moving size is
  65 (64 v-features + a ones column = softmax denominator).  This halves
  PV cost vs. the [65, 512] orientation.
- RoPE rotate-half: front-critical tiles get it as a SECOND projection
  with row-swapped weights (wqkT slices 8/9, PE is idle early); steady
  state uses DVE stream_shuffle copies (partition-offset views).  The
  sign of the rotated term is baked into the sin table.
- Attention out [q, hd] is normalized by a per-partition scalar
  (1/denominator), then transposed feat-major via identity matmuls so
  the output projection can contract features on partitions.
- exp on ACT from 3-bank/2-bank PSUM groups; ACT (~255us busy) is the
  bottleneck engine and everything else is scheduled to hide under it.
- Projections / ropes / out-projection stream into the bursts as PE
  filler via a named pending-closure queue; require() guarantees
  writers are always emitted before readers, V projection is emitted
  inline just-in-time, and dummy matmuls keep the PE p-state warm when
  filler runs out.
"""
import sys

sys.path.insert(0, "/opt/trn_rl_repo")
import numpy as np
import concourse.bass as bass
import concourse.bacc as bacc
import concourse.mybir as mybir
from concourse.tile import TileContext
from concourse.bass_utils import run_bass_kernel_spmd

L = 2048          # sequence length
D = 1024          # model dim
HD = 64           # head dim
KT = L // 128     # 16 k position tiles
QC = 4            # q chunks of 512
DT = mybir.dt.bfloat16
F32 = mybir.dt.float32
F8 = mybir.dt.float8e4
SCALE = HD ** -0.5
AF = mybir.ActivationFunctionType
DR = mybir.MatmulPerfMode.DoubleRow

N_WARM = 75       # PE p-state warmup matmuls (until first wave matmul)


def build_nc():
    nc = bacc.Bacc("TRN2", target_bir_lowering=False, debug=False, num_devices=8)
    xT = nc.dram_tensor("xT", [D, L], DT, kind="ExternalInput")
    # slices 0-7: q/k projection weights; 8/9: row-swapped fc0/fc4
    wqkT = nc.dram_tensor("wqkT", [10, 128, 8, 128], DT, kind="ExternalInput")
    wvT = nc.dram_tensor("wvT", [D, 512], DT, kind="ExternalInput")
    bqk = nc.dram_tensor("bqk", [128, 10], F32, kind="ExternalInput")
    bv = nc.dram_tensor("bv", [1, 512], DT, kind="ExternalInput")
    woT = nc.dram_tensor("woT", [512, D], DT, kind="ExternalInput")
    cosT = nc.dram_tensor("cosT", [128, L], DT, kind="ExternalInput")
    sinT = nc.dram_tensor("sinT", [128, L], DT, kind="ExternalInput")
    ident = nc.dram_tensor("ident", [128, 128], DT, kind="ExternalInput")
    zeros8 = nc.dram_tensor("zeros8", [128, L], F8, kind="ExternalInput")
    out = nc.dram_tensor("out", [L, D], F32, kind="ExternalOutput")

    with TileContext(nc) as tc:
        with (
            tc.tile_pool(name="const", bufs=1) as cp,
            tc.tile_pool(name="wstream", bufs=1) as wsp,
            tc.tile_pool(name="rb", bufs=6) as rbp,
            tc.tile_pool(name="rtmp", bufs=2) as rtp,
            tc.tile_pool(name="exps", bufs=2) as ep,
            tc.tile_pool(name="apair", bufs=2) as app,
            tc.tile_pool(name="ctile", bufs=2) as ctp,
            tc.tile_pool(name="osb", bufs=3) as osp,
            tc.tile_pool(name="small", bufs=2) as sp,
            tc.tile_pool(name="psum", bufs=1, space="PSUM") as pp,
        ):
            dma = nc.default_dma_engine     # SP / HWDGE
            dma2 = nc.gpsimd                # Pool / SWDGE

            scratch = cp.tile([1, 640], DT)
            nc.vector.memset(scratch[:], 0.0)

            wqk_tiles = {}

            def fetch_wqk(fc, eng=dma):
                t = wsp.tile([128, 8, 128], DT, tag=f"wqk{fc}", name=f"wqk{fc}")
                eng.dma_start(out=t[:], in_=wqkT[fc])
                wqk_tiles[fc] = t

            # qkT8: data slices 0-7 (fc order), shared zero slice at 8.
            qkT8 = cp.tile([128, 9, L], F8)
            xT_sb = cp.tile([128, 8, L], DT)

            # DMA order: xT + the four wave weight slices first.
            dma.dma_start(out=xT_sb[:, 0, :], in_=xT[0:128, :])
            dma2.dma_start(out=xT_sb[:, 1, :], in_=xT[128:256, :])
            fetch_wqk(0, dma)
            fetch_wqk(8, dma)
            fetch_wqk(4, dma2)
            fetch_wqk(9, dma2)
            for c in range(2, 8):
                (dma, dma2)[c % 2].dma_start(out=xT_sb[:, c, :],
                                             in_=xT[c * 128:(c + 1) * 128, :])
            bqk_sb = cp.tile([128, 10], F32)
            dma2.dma_start(out=bqk_sb[:], in_=bqk[:])
            dma.dma_start(out=qkT8[:, 8, :], in_=zeros8[:])
            cos_sb = cp.tile([128, L], DT)
            dma.dma_start(out=cos_sb[:], in_=cosT[:])
            sin_sb = cp.tile([128, L], DT)
            dma.dma_start(out=sin_sb[:], in_=sinT[:])
            ident_sb = cp.tile([128, 128], DT)
            dma2.dma_start(out=ident_sb[:], in_=ident[:])
            bv_sb = cp.tile([1, 512], DT)
            dma2.dma_start(out=bv_sb[:], in_=bv[:])
            bv_bc = cp.tile([128, 512], DT)
            nc.gpsimd.partition_broadcast(bv_bc[:], bv_sb[:])
            wvT_sb = cp.tile([128, 8, 512], DT)
            woT_sb = cp.tile([128, 4, D], DT)

            V_sb = cp.tile([128, KT, 8 * (HD + 1)], DT)
            v4 = V_sb[:].rearrange("p k (h c) -> p k h c", c=HD + 1)
            nc.vector.memset(v4[:, :, :, HD:HD + 1], 1.0)

            # ---- PE warmup (p-state ramp) until first wave matmul ----
            warm = pp.tile([128, 128], F32, tag="ot", bufs=1, name="warm")
            for _ in range(N_WARM):
                nc.tensor.matmul(warm[:], lhsT=scratch[0:1, 0:128],
                                 rhs=scratch[0:1, 128:256], start=True, stop=True)

            def zero_bank(ap):
                nc.tensor.matmul(ap, lhsT=scratch[0:1, 0:128],
                                 rhs=scratch[0:1, 128:128 + ap.shape[-1]],
                                 start=True, stop=False, skip_group_check=True)

            def dummy_fill(n):
                dm = pp.tile([128, 256], F32, tag="ot", bufs=1, name="dm")
                for _ in range(n):
                    nc.tensor.matmul(dm[:], lhsT=scratch[0:1, 0:128],
                                     rhs=scratch[0:1, 128:384],
                                     start=True, stop=True)

            rb_tiles = {}

            def rb_tile(fc):
                if fc not in rb_tiles:
                    rb_tiles[fc] = rbp.tile([128, L], DT, tag="rb", name=f"rb{fc}")
                return rb_tiles[fc]

            def rope_dve(fc, nt, rot):
                """qkT8[fc] <- rb*cos + rot*sin for positions nt*512.."""
                rb = rb_tiles[fc]
                ntr = slice(nt * 512, (nt + 1) * 512)
                tmp = rtp.tile([128, 512], DT, tag="rtmp", name=f"rm{fc}_{nt}")
                nc.vector.tensor_mul(tmp[:], rb[:, ntr], cos_sb[:, ntr])
                nc.vector.tensor_mul(rot, rot, sin_sb[:, ntr])
                nc.vector.tensor_add(qkT8[:, fc, ntr], tmp[:], rot)

            def rope_shuffle(fc, nt):
                """rotate-half via DVE stream_shuffle (steady-state path)."""
                rb = rb_tiles[fc]
                ntr = slice(nt * 512, (nt + 1) * 512)
                rot = rtp.tile([128, 512], DT, tag="rot", name=f"rt{fc}_{nt}")
                idm = list(range(32))
                for h2 in range(2):
                    p = 64 * h2
                    nc.vector.stream_shuffle(rot[p:p + 32, :],
                                             rb[p + 32:p + 64, ntr], idm)
                    nc.vector.stream_shuffle(rot[p + 32:p + 64, :],
                                             rb[p:p + 32, ntr], idm)
                rope_dve(fc, nt, rot[:])

            def bias_to(dst, acc, col):
                nc.vector.tensor_scalar_add(dst, acc, bqk_sb[:, col:col + 1])

            def qk_proj_now(fc, nt):
                ot = pp.tile([128, 512], F32, tag="ot", bufs=1, name=f"qp{fc}_{nt}")
                for kc in range(8):
                    nc.tensor.matmul(ot[:], lhsT=wqk_tiles[fc][:, kc, :],
                                     rhs=xT_sb[:, kc, nt * 512:(nt + 1) * 512],
                                     start=(kc == 0), stop=(kc == 7))
                bias_to(rb_tile(fc)[:, nt * 512:(nt + 1) * 512], ot[:], fc)

            def rope_rotproj(fc, nt, rot_acc=None):
                """rotate-half via a projection with row-swapped weights."""
                if rot_acc is None:
                    ra = pp.tile([128, 512], F32, tag="ot", bufs=1,
                                 name=f"qr{fc}_{nt}")
                    wr = wqk_tiles[8 if fc == 0 else 9]
                    for kc in range(8):
                        nc.tensor.matmul(
                            ra[:], lhsT=wr[:, kc, :],
                            rhs=xT_sb[:, kc, nt * 512:(nt + 1) * 512],
                            start=(kc == 0), stop=(kc == 7))
                    rot_acc = ra[:]
                rot = rtp.tile([128, 512], DT, tag="rot", name=f"rr{fc}_{nt}")
                bias_to(rot[:], rot_acc, 8 if fc == 0 else 9)
                rope_dve(fc, nt, rot[:])

            # ---- wave: kc-outer, 6 accumulators: q/k nt0 of heads 0-1
            #      (+ their rotated projections) and k nt1-2 ----
            spA = pp.tile([128, 3, 512], F32, tag="sA", name="waveA")
            spB = pp.tile([128, 3, 512], F32, tag="sB", name="waveB")
            wave = [(0, 0, spA[:, 0, :]), (8, 0, spA[:, 1, :]),
                    (4, 0, spA[:, 2, :]), (9, 0, spB[:, 0, :]),
                    (4, 1, spB[:, 1, :]), (4, 2, spB[:, 2, :])]
            for kc in range(8):
                for fc, nt, acc in wave:
                    nc.tensor.matmul(acc, lhsT=wqk_tiles[fc][:, kc, :],
                                     rhs=xT_sb[:, kc, nt * 512:(nt + 1) * 512],
                                     start=(kc == 0), stop=(kc == 7))
            accs = {(fc, nt): acc for fc, nt, acc in wave}
            # q/k nt0 units via the rotated projections
            bias_to(rb_tile(0)[:, 0:512], accs[(0, 0)], 0)
            rope_rotproj(0, 0, rot_acc=accs[(8, 0)])
            bias_to(rb_tile(4)[:, 0:512], accs[(4, 0)], 4)
            rope_rotproj(4, 0, rot_acc=accs[(9, 0)])
            bias_to(rb_tile(4)[:, 512:1024], accs[(4, 1)], 4)
            bias_to(rb_tile(4)[:, 1024:1536], accs[(4, 2)], 4)
            # k nt1-2 rot tiles (PE is free while early bursts are ACT-bound)
            rope_rotproj(4, 1)
            rope_rotproj(4, 2)

            # ---- named filler queue ----
            pending = []     # list of (name, closure)
            done = {"r0_0", "r4_0", "r4_1", "r4_2"}

            def run_next():
                name, fn = pending.pop(0)
                fn()
                done.add(name)

            def drain(n):
                for _ in range(min(n, len(pending))):
                    run_next()

            def require(name):
                if name in done:
                    return
                assert any(n == name for n, _ in pending), f"missing {name}"
                while name not in done:
                    run_next()

            def qk_proj(fc, nt):
                return (f"qp{fc}_{nt}", lambda: qk_proj_now(fc, nt))

            def rope_f(fc, nt):
                return (f"r{fc}_{nt}", lambda: rope_shuffle(fc, nt))

            def rope_rp(fc, nt):
                return (f"r{fc}_{nt}", lambda: rope_rotproj(fc, nt))

            def fetch_f(fc):
                return (f"fw{fc}", lambda: fetch_wqk(fc))

            def fetch_wv():
                for c in range(8):
                    (dma, dma2)[c % 2].dma_start(
                        out=wvT_sb[:, c, :], in_=wvT[c * 128:(c + 1) * 128, :])

            def fetch_wo():
                for c in range(4):
                    dma2.dma_start(out=woT_sb[:, c, :],
                                   in_=woT[c * 128:(c + 1) * 128, :])

            def kchain(ch2):
                fk, fq = 4 + ch2, ch2
                items = [fetch_f(fk)]
                for nt in range(4):
                    items += [qk_proj(fk, nt), rope_f(fk, nt)]
                items += [fetch_f(fq)]
                for nt in range(4):
                    items += [qk_proj(fq, nt), rope_f(fq, nt)]
                return items

            q0chain = []
            for nt in range(1, 4):
                q0chain += [qk_proj(0, nt), rope_f(0, nt)]

            pending.extend(
                [qk_proj(4, 3), rope_rp(4, 3), ("fwv", fetch_wv)] +
                q0chain + kchain(1) + [("fwo", fetch_wo)] +
                kchain(2) + kchain(3))

            # ---- V projection: emitted inline, just in time ----
            v_done = set()

            def v_need(lt, hp):
                if (lt, hp) in v_done:
                    return
                require("fwv")
                v_done.add((lt, hp))
                ot = pp.tile([128, 128], F32, tag="ot", bufs=1, name=f"vp{lt}_{hp}")
                for kc in range(8):
                    nc.tensor.matmul(
                        ot[:],
                        lhsT=xT_sb[:, kc, lt * 128:(lt + 1) * 128],
                        rhs=wvT_sb[:, kc, hp * 128:(hp + 1) * 128],
                        start=(kc == 0), stop=(kc == 7))
                nc.vector.tensor_add(
                    v4[:, lt, 2 * hp:2 * hp + 2, 0:HD],
                    ot[:].rearrange("p (h c) -> p h c", c=HD),
                    bv_bc[:, hp * 128:(hp + 1) * 128]
                    .rearrange("p (h c) -> p h c", c=HD))

            # ---- attention bursts ----
            GROUPS = [(0, 3, "sA"), (3, 3, "sB"), (6, 3, "sA"),
                      (9, 3, "sB"), (12, 2, "sA"), (14, 2, "sB")]
            KROPE_NT = {0: 0, 3: 1, 6: 2, 9: 2, 12: 3, 14: 3}

            def out_proj(cT, qc, dt_, mq):
                def emit():
                    require("fwo")
                    ops = pp.tile([128, 512], F32, tag="ot", bufs=1,
                                  name=f"op{qc}{dt_}{mq}")
                    for cc in range(4):
                        nc.tensor.matmul(ops[:], lhsT=cT[:, cc, qc, mq, :],
                                         rhs=woT_sb[:, cc, dt_ * 512:(dt_ + 1) * 512],
                                         start=(cc == 0), stop=(cc == 3))
                    o = osp.tile([128, 512], F32, tag="osb", name=f"os{qc}{dt_}{mq}")
                    nc.vector.tensor_copy(o[:], ops[:])
                    dma.dma_start(
                        out=out[qc * 512 + mq * 128: qc * 512 + (mq + 1) * 128,
                                dt_ * 512:(dt_ + 1) * 512],
                        in_=o[:])
                return (f"op{qc}_{dt_}_{mq}", emit)

            cT = ctp.tile([128, 4, QC, 4, 128], DT, tag="cT", bufs=1, name="cT")
            apair_box = [None]

            def burst(h, qc):
                ch2 = h // 2
                prow = slice(64 * (h % 2), 64 * (h % 2) + 64)
                eq, ek = ch2, 4 + ch2
                require(f"r{ch2}_{qc}")
                exp_t = ep.tile([128, KT, 512], DT, tag="exp", name=f"ex{qc}{h}")
                pv = pp.tile([128, 512], F32, tag="pvx", name=f"pv{qc}{h}")
                zero_bank(pv[:])
                pvv = pv[:, 0:320].rearrange("p (q c) -> p q c", c=80)
                if h % 2 == 0:
                    apair_box[0] = app.tile([128, 4, 2, HD], DT, tag="ap",
                                            name=f"ap{qc}{h}")
                apair = apair_box[0]

                def pv_group(kt0, n):
                    for lt in range(kt0, kt0 + n):
                        v_need(lt, ch2)
                    for i in range(n):
                        kt = kt0 + i
                        for qt in range(4):
                            nc.tensor.matmul(
                                pvv[:, qt, 0:65],
                                lhsT=exp_t[:, kt, qt * 128:(qt + 1) * 128],
                                rhs=V_sb[:, kt, h * 65:(h + 1) * 65],
                                start=False, stop=(kt == KT - 1),
                                skip_group_check=True)

                prev = None
                for gi, (kt0, n, tag) in enumerate(GROUPS):
                    require(f"r{4 + ch2}_{KROPE_NT[kt0]}")
                    sg = pp.tile([128, n, 512], F32, tag=tag,
                                 name=f"sg{qc}{h}{kt0}")
                    for i in range(n):
                        kt = kt0 + i
                        nc.tensor.matmul(
                            sg[:, i, :],
                            lhsT=qkT8[prow, ek:9:8 - ek, kt * 128:(kt + 1) * 128],
                            rhs=qkT8[prow, eq:9:8 - eq, qc * 512:(qc + 1) * 512],
                            start=True, stop=True, perf_mode=DR)
                    nc.scalar.activation(
                        exp_t[:, kt0:kt0 + n, :].rearrange("p a b -> p (a b)"),
                        sg[:].rearrange("p a b -> p (a b)"), AF.Exp, scale=SCALE)
                    if gi in (1, 3, 4):
                        if pending:
                            drain(1)
                        else:
                            dummy_fill(3)
                    if prev is not None:
                        pv_group(prev[0], prev[1])
                    prev = (kt0, n)
                pv_group(prev[0], prev[1])

                r = sp.tile([128, 4], F32, tag="rsb", name=f"r{qc}{h}")
                nc.vector.reciprocal(
                    r[:], pvv[:, :, 64:65].rearrange("p q c -> p (q c)"))
                for qt in range(4):
                    nc.vector.tensor_scalar_mul(
                        apair[:, qt, h % 2, :],
                        pvv[:, qt, 0:64], r[:, qt:qt + 1])

                if h % 2 == 1:
                    xp = pp.tile([128, 4, 128], F32, tag="pvx",
                                 name=f"xp{qc}{ch2}")
                    zero_bank(xp[:].rearrange("p a b -> p (a b)"))
                    for qt in range(4):
                        nc.tensor.matmul(
                            xp[:, qt, :],
                            lhsT=apair[:, qt, :, :].rearrange("p a b -> p (a b)"),
                            rhs=ident_sb[:], start=False, stop=True,
                            skip_group_check=True)
                    nc.vector.tensor_copy(cT[:, ch2, qc, :, :], xp[:])
                    if pending:
                        drain(1)

            for hp in range(4):
                for qc in range(QC):
                    burst(2 * hp, qc)
                    burst(2 * hp + 1, qc)
                    if hp == 3 and qc < QC - 1:
                        pending.extend(out_proj(cT, qc, dt_, mq)
                                       for dt_ in range(2) for mq in range(4))
            while pending:
                run_next()
            # flush: qc3 out-proj across all now-idle psum banks in parallel
            require("fwo")
            fA = pp.tile([128, 3, 512], F32, tag="sA", name="fA")
            fB = pp.tile([128, 3, 512], F32, tag="sB", name="fB")
            fO = pp.tile([128, 512], F32, tag="ot", bufs=1, name="fO")
            fP = pp.tile([128, 512], F32, tag="pvx", name="fP")
            banks = [fA[:, 0, :], fA[:, 1, :], fA[:, 2, :],
                     fB[:, 0, :], fB[:, 1, :], fB[:, 2, :], fO[:], fP[:]]
            qc3 = QC - 1
            fl = [(dt_, mq) for dt_ in range(2) for mq in range(4)]
            for (dt_, mq), bank in zip(fl, banks):
                for cc in range(4):
                    nc.tensor.matmul(bank, lhsT=cT[:, cc, qc3, mq, :],
                                     rhs=woT_sb[:, cc, dt_ * 512:(dt_ + 1) * 512],
                                     start=(cc == 0), stop=(cc == 3))
            for (dt_, mq), bank in zip(fl, banks):
                o = osp.tile([128, 512], F32, tag="osb", name=f"fs{dt_}{mq}")
                nc.vector.tensor_copy(o[:], bank)
                dma.dma_start(
                    out=out[qc3 * 512 + mq * 128: qc3 * 512 + (mq + 1) * 128,
                            dt_ * 512:(dt_ + 1) * 512],
                    in_=o[:])
    nc.compile()
    return nc


def _rope_tables_np():
    import ml_dtypes
    bf16 = ml_dtypes.bfloat16
    inv_freq = 1.0 / (10000.0 ** (np.arange(0, HD, 2, dtype=np.float32) / HD))
    t = np.arange(L, dtype=np.float32)
    freqs = np.outer(t, inv_freq).astype(np.float32)       # [L, 32]
    cos_h = np.cos(freqs).T                                # [32, L]
    sin_h = np.sin(freqs).T
    cosT = np.concatenate([cos_h, cos_h], 0)               # [64, L]
    sinT = np.concatenate([-sin_h, sin_h], 0)              # sign baked for rot trick
    return (np.tile(cosT, (2, 1)).astype(bf16),
            np.tile(sinT, (2, 1)).astype(bf16))            # [128, L]


_NC_CACHE = {}


def kernel(x, w_qkv, b_qkv, w_out, b_out):
    import ml_dtypes
    bf16 = ml_dtypes.bfloat16
    f8 = ml_dtypes.float8_e4m3
    if "nc" not in _NC_CACHE:
        _NC_CACHE["nc"] = build_nc()
    nc = _NC_CACHE["nc"]

    cosT, sinT = _rope_tables_np()
    ident = np.eye(128, dtype=np.float32).astype(bf16)
    zeros8 = np.zeros((128, L), dtype=f8)
    p = np.arange(128)
    swap = (p // 64) * 64 + ((p % 64) + 32) % 64
    in_maps = []
    for core in range(8):
        b, g = divmod(core, 2)
        s = slice(512 * g, 512 * (g + 1))
        wqk = np.concatenate([w_qkv[0:D][s], w_qkv[D:2 * D][s]], 0)  # [1024, 1024]
        bqk_v = np.concatenate([b_qkv[0:D][s], b_qkv[D:2 * D][s]])
        # [8, 128, 8, 128] = [fc, dpart, kc, m]
        wqk_t = wqk.T.reshape(8, 128, 8, 128).transpose(2, 1, 0, 3)
        # swapped-row variants of fc0 / fc4 (rotate-half projections);
        # m indexes output features, so permute the last axis
        wswap = np.stack([wqk_t[0][:, :, swap], wqk_t[4][:, :, swap]])
        bqk_cols = np.concatenate(
            [bqk_v.reshape(8, 128).T,
             bqk_v.reshape(8, 128)[[0, 4]].T[swap]], axis=1)   # [128, 10]
        in_maps.append({
            "xT": np.ascontiguousarray(x[b].T).astype(bf16),
            "wqkT": np.ascontiguousarray(
                np.concatenate([wqk_t, wswap], 0)).astype(bf16),
            "wvT": np.ascontiguousarray(w_qkv[2 * D:3 * D][s].T).astype(bf16),
            "bqk": np.ascontiguousarray(bqk_cols).astype(np.float32),
            "bv": b_qkv[2 * D:3 * D][s][None].astype(bf16),
            "woT": np.ascontiguousarray(w_out[:, s].T).astype(bf16),
            "cosT": cosT,
            "sinT": sinT,
            "ident": ident,
            "zeros8": zeros8,
        })
    res = run_bass_kernel_spmd(nc, in_maps, list(range(8)))
    _NC_CACHE["last_results"] = res
    parts = [r["out"] for r in res.results]
    full = np.stack([parts[2 * b] + parts[2 * b + 1] for b in range(4)])
    return (full + b_out[None, None, :]).astype(np.float32)


# revision 34
# speedup vs baseline: 1.0938x; 1.0094x over previous
"""Trainium2 Bass kernel for multi-head attention (B=4, L=2048, D=1024, H=16).

Sharding: 8 cores = 4 batches x 2 head-groups (8 heads each).

Design notes (cost-model-driven):
- Scores matmuls run in fp8-e4m3 with DoubleRow perf mode.  Each head's
  64 features sit on a 64-partition window (2 heads per 128 partitions,
  bases 0/64); the DR slot pair is (real features, shared zero slice)
  addressed by a step-sliced AP, so one DR matmul contracts the whole
  head at 0.5 cycles/row -- half the bf16 cost.
- PV runs in bf16 with q on the OUTPUT PARTITIONS: out [128 q, 65] with
  exp as the stationary operand, so the per-instruction moving size is
  65 (64 v-features + a ones column = softmax denominator).  This halves
  PV cost vs. the [65, 512] orientation.
- RoPE rotate-half: front-critical tiles get it as a SECOND projection
  with row-swapped weights (wqkT slices 8/9, PE is idle early); steady
  state uses DVE stream_shuffle copies (partition-offset views).  The
  sign of the rotated term is baked into the sin table.
- Attention out [q, hd] is normalized by a per-partition scalar
  (1/denominator), then transposed feat-major via identity matmuls so
  the output projection can contract features on partitions.
- exp on ACT from 3-bank/2-bank PSUM groups; ACT (~255us busy) is the
  bottleneck engine and everything else is scheduled to hide under it.
- Projections / ropes / out-projection stream into the bursts as PE
  filler via a named pending-closure queue; require() guarantees
  writers are always emitted before readers, V projection is emitted
  inline just-in-time, and dummy matmuls keep the PE p-state warm when
  filler runs out.
"""
import sys

sys.path.insert(0, "/opt/trn_rl_repo")
import numpy as np
import concourse.bass as bass
import concourse.bacc as bacc
import concourse.mybir as mybir
from concourse.tile import TileContext
from concourse.bass_utils import run_bass_kernel_spmd

L = 2048          # sequence length
D = 1024          # model dim
HD = 64           # head dim
KT = L // 128     # 16 k position tiles
QC = 4            # q chunks of 512
DT = mybir.dt.bfloat16
F32 = mybir.dt.float32
F8 = mybir.dt.float8e4
SCALE = HD ** -0.5
AF = mybir.ActivationFunctionType
DR = mybir.MatmulPerfMode.DoubleRow

N_WARM = 75       # PE p-state warmup matmuls (until first wave matmul)


def build_nc():
    nc = bacc.Bacc("TRN2", target_bir_lowering=False, debug=False, num_devices=8)
    xT = nc.dram_tensor("xT", [D, L], DT, kind="ExternalInput")
    # slices 0-7: q/k projection weights; 8/9: row-swapped fc0/fc4
    wqkT = nc.dram_tensor("wqkT", [10, 128, 8, 128], DT, kind="ExternalInput")
    wvT = nc.dram_tensor("wvT", [D, 512], DT, kind="ExternalInput")
    bqk = nc.dram_tensor("bqk", [128, 10], F32, kind="ExternalInput")
    bv = nc.dram_tensor("bv", [1, 512], DT, kind="ExternalInput")
    woT = nc.dram_tensor("woT", [512, D], DT, kind="ExternalInput")
    cosT = nc.dram_tensor("cosT", [128, L], DT, kind="ExternalInput")
    sinT = nc.dram_tensor("sinT", [128, L], DT, kind="ExternalInput")
    ident = nc.dram_tensor("ident", [128, 128], DT, kind="ExternalInput")
    zeros8 = nc.dram_tensor("zeros8", [128, L], F8, kind="ExternalInput")
    out = nc.dram_tensor("out", [L, D], F32, kind="ExternalOutput")

    with TileContext(nc) as tc:
        with (
            tc.tile_pool(name="const", bufs=1) as cp,
            tc.tile_pool(name="wstream", bufs=1) as wsp,
            tc.tile_pool(name="rb", bufs=6) as rbp,
            tc.tile_pool(name="rtmp", bufs=2) as rtp,
            tc.tile_pool(name="exps", bufs=2) as ep,
            tc.tile_pool(name="apair", bufs=2) as app,
            tc.tile_pool(name="ctile", bufs=2) as ctp,
            tc.tile_pool(name="osb", bufs=6) as osp,
            tc.tile_pool(name="small", bufs=2) as sp,
            tc.tile_pool(name="psum", bufs=1, space="PSUM") as pp,
        ):
            dma = nc.default_dma_engine     # SP / HWDGE
            dma2 = nc.gpsimd                # Pool / SWDGE

            scratch = cp.tile([1, 640], DT)
            nc.vector.memset(scratch[:], 0.0)

            wqk_tiles = {}

            def fetch_wqk(fc, eng=dma):
                t = wsp.tile([128, 8, 128], DT, tag=f"wqk{fc}", name=f"wqk{fc}")
                eng.dma_start(out=t[:], in_=wqkT[fc])
                wqk_tiles[fc] = t

            # qkT8: data slices 0-7 (fc order), shared zero slice at 8.
            qkT8 = cp.tile([128, 9, L], F8)
            xT_sb = cp.tile([128, 8, L], DT)

            # DMA order: xT + the four wave weight slices first.
            dma.dma_start(out=xT_sb[:, 0, :], in_=xT[0:128, :])
            dma2.dma_start(out=xT_sb[:, 1, :], in_=xT[128:256, :])
            fetch_wqk(0, dma)
            fetch_wqk(8, dma)
            fetch_wqk(4, dma2)
            fetch_wqk(9, dma2)
            for c in range(2, 8):
                (dma, dma2)[c % 2].dma_start(out=xT_sb[:, c, :],
                                             in_=xT[c * 128:(c + 1) * 128, :])
            bqk_sb = cp.tile([128, 10], F32)
            dma2.dma_start(out=bqk_sb[:], in_=bqk[:])
            dma.dma_start(out=qkT8[:, 8, :], in_=zeros8[:])
            cos_sb = cp.tile([128, L], DT)
            dma.dma_start(out=cos_sb[:], in_=cosT[:])
            sin_sb = cp.tile([128, L], DT)
            dma.dma_start(out=sin_sb[:], in_=sinT[:])
            ident_sb = cp.tile([128, 128], DT)
            dma2.dma_start(out=ident_sb[:], in_=ident[:])
            bv_sb = cp.tile([1, 512], DT)
            dma2.dma_start(out=bv_sb[:], in_=bv[:])
            bv_bc = cp.tile([128, 512], DT)
            nc.gpsimd.partition_broadcast(bv_bc[:], bv_sb[:])
            wvT_sb = cp.tile([128, 8, 512], DT)
            woT_sb = cp.tile([128, 4, D], DT)

            V_sb = cp.tile([128, KT, 8 * (HD + 1)], DT)
            v4 = V_sb[:].rearrange("p k (h c) -> p k h c", c=HD + 1)
            nc.vector.memset(v4[:, :, :, HD:HD + 1], 1.0)

            # ---- PE warmup (p-state ramp) until first wave matmul ----
            warm = pp.tile([128, 128], F32, tag="ot", bufs=1, name="warm")
            for _ in range(N_WARM):
                nc.tensor.matmul(warm[:], lhsT=scratch[0:1, 0:128],
                                 rhs=scratch[0:1, 128:256], start=True, stop=True)

            def zero_bank(ap):
                nc.tensor.matmul(ap, lhsT=scratch[0:1, 0:128],
                                 rhs=scratch[0:1, 128:128 + ap.shape[-1]],
                                 start=True, stop=False, skip_group_check=True)

            def dummy_fill(n):
                dm = pp.tile([128, 256], F32, tag="ot", bufs=1, name="dm")
                for _ in range(n):
                    nc.tensor.matmul(dm[:], lhsT=scratch[0:1, 0:128],
                                     rhs=scratch[0:1, 128:384],
                                     start=True, stop=True)

            rb_tiles = {}

            def rb_tile(fc):
                if fc not in rb_tiles:
                    rb_tiles[fc] = rbp.tile([128, L], DT, tag="rb", name=f"rb{fc}")
                return rb_tiles[fc]

            def rope_dve(fc, nt, rot):
                """qkT8[fc] <- rb*cos + rot*sin for positions nt*512.."""
                rb = rb_tiles[fc]
                ntr = slice(nt * 512, (nt + 1) * 512)
                tmp = rtp.tile([128, 512], DT, tag="rtmp", name=f"rm{fc}_{nt}")
                nc.vector.tensor_mul(tmp[:], rb[:, ntr], cos_sb[:, ntr])
                nc.vector.tensor_mul(rot, rot, sin_sb[:, ntr])
                nc.vector.tensor_add(qkT8[:, fc, ntr], tmp[:], rot)

            def rope_shuffle(fc, nt):
                """rotate-half via DVE stream_shuffle (steady-state path)."""
                rb = rb_tiles[fc]
                ntr = slice(nt * 512, (nt + 1) * 512)
                rot = rtp.tile([128, 512], DT, tag="rot", name=f"rt{fc}_{nt}")
                idm = list(range(32))
                for h2 in range(2):
                    p = 64 * h2
                    nc.vector.stream_shuffle(rot[p:p + 32, :],
                                             rb[p + 32:p + 64, ntr], idm)
                    nc.vector.stream_shuffle(rot[p + 32:p + 64, :],
                                             rb[p:p + 32, ntr], idm)
                rope_dve(fc, nt, rot[:])

            def bias_to(dst, acc, col):
                nc.vector.tensor_scalar_add(dst, acc, bqk_sb[:, col:col + 1])

            def qk_proj_now(fc, nt):
                ot = pp.tile([128, 512], F32, tag="ot", bufs=1, name=f"qp{fc}_{nt}")
                for kc in range(8):
                    nc.tensor.matmul(ot[:], lhsT=wqk_tiles[fc][:, kc, :],
                                     rhs=xT_sb[:, kc, nt * 512:(nt + 1) * 512],
                                     start=(kc == 0), stop=(kc == 7))
                bias_to(rb_tile(fc)[:, nt * 512:(nt + 1) * 512], ot[:], fc)

            def rope_rotproj(fc, nt, rot_acc=None):
                """rotate-half via a projection with row-swapped weights."""
                if rot_acc is None:
                    ra = pp.tile([128, 512], F32, tag="ot", bufs=1,
                                 name=f"qr{fc}_{nt}")
                    wr = wqk_tiles[8 if fc == 0 else 9]
                    for kc in range(8):
                        nc.tensor.matmul(
                            ra[:], lhsT=wr[:, kc, :],
                            rhs=xT_sb[:, kc, nt * 512:(nt + 1) * 512],
                            start=(kc == 0), stop=(kc == 7))
                    rot_acc = ra[:]
                rot = rtp.tile([128, 512], DT, tag="rot", name=f"rr{fc}_{nt}")
                bias_to(rot[:], rot_acc, 8 if fc == 0 else 9)
                rope_dve(fc, nt, rot[:])

            # ---- wave: kc-outer, 6 accumulators: q/k nt0 of heads 0-1
            #      (+ their rotated projections) and k nt1-2 ----
            spA = pp.tile([128, 3, 512], F32, tag="sA", name="waveA")
            spB = pp.tile([128, 3, 512], F32, tag="sB", name="waveB")
            wave = [(0, 0, spA[:, 0, :]), (8, 0, spA[:, 1, :]),
                    (4, 0, spA[:, 2, :]), (9, 0, spB[:, 0, :]),
                    (4, 1, spB[:, 1, :]), (4, 2, spB[:, 2, :])]
            for kc in range(8):
                for fc, nt, acc in wave:
                    nc.tensor.matmul(acc, lhsT=wqk_tiles[fc][:, kc, :],
                                     rhs=xT_sb[:, kc, nt * 512:(nt + 1) * 512],
                                     start=(kc == 0), stop=(kc == 7))
            accs = {(fc, nt): acc for fc, nt, acc in wave}
            # q/k nt0 units via the rotated projections
            bias_to(rb_tile(0)[:, 0:512], accs[(0, 0)], 0)
            rope_rotproj(0, 0, rot_acc=accs[(8, 0)])
            bias_to(rb_tile(4)[:, 0:512], accs[(4, 0)], 4)
            rope_rotproj(4, 0, rot_acc=accs[(9, 0)])
            bias_to(rb_tile(4)[:, 512:1024], accs[(4, 1)], 4)
            bias_to(rb_tile(4)[:, 1024:1536], accs[(4, 2)], 4)
            # k nt1-2 rot tiles (PE is free while early bursts are ACT-bound)
            rope_rotproj(4, 1)
            rope_rotproj(4, 2)

            # ---- named filler queue ----
            pending = []     # list of (name, closure)
            done = {"r0_0", "r4_0", "r4_1", "r4_2"}

            def run_next():
                name, fn = pending.pop(0)
                fn()
                done.add(name)

            def drain(n):
                for _ in range(min(n, len(pending))):
                    run_next()

            def require(name):
                if name in done:
                    return
                assert any(n == name for n, _ in pending), f"missing {name}"
                while name not in done:
                    run_next()

            def qk_proj(fc, nt):
                return (f"qp{fc}_{nt}", lambda: qk_proj_now(fc, nt))

            def rope_f(fc, nt):
                return (f"r{fc}_{nt}", lambda: rope_shuffle(fc, nt))

            def rope_rp(fc, nt):
                return (f"r{fc}_{nt}", lambda: rope_rotproj(fc, nt))

            def fetch_f(fc):
                return (f"fw{fc}", lambda: fetch_wqk(fc))

            def fetch_wv():
                for c in range(8):
                    (dma, dma2)[c % 2].dma_start(
                        out=wvT_sb[:, c, :], in_=wvT[c * 128:(c + 1) * 128, :])

            def fetch_wo():
                for c in range(4):
                    dma2.dma_start(out=woT_sb[:, c, :],
                                   in_=woT[c * 128:(c + 1) * 128, :])

            def kchain(ch2):
                fk, fq = 4 + ch2, ch2
                items = [fetch_f(fk)]
                for nt in range(4):
                    items += [qk_proj(fk, nt), rope_f(fk, nt)]
                items += [fetch_f(fq)]
                for nt in range(4):
                    items += [qk_proj(fq, nt), rope_f(fq, nt)]
                return items

            q0chain = []
            for nt in range(1, 4):
                q0chain += [qk_proj(0, nt), rope_f(0, nt)]

            pending.extend(
                [qk_proj(4, 3), rope_rp(4, 3), ("fwv", fetch_wv)] +
                q0chain + kchain(1) + [("fwo", fetch_wo)] +
                kchain(2) + kchain(3))

            # ---- V projection: emitted inline, just in time ----
            v_done = set()

            def v_need(lt, hp):
                if (lt, hp) in v_done:
                    return
                require("fwv")
                v_done.add((lt, hp))
                ot = pp.tile([128, 128], F32, tag="ot", bufs=1, name=f"vp{lt}_{hp}")
                for kc in range(8):
                    nc.tensor.matmul(
                        ot[:],
                        lhsT=xT_sb[:, kc, lt * 128:(lt + 1) * 128],
                        rhs=wvT_sb[:, kc, hp * 128:(hp + 1) * 128],
                        start=(kc == 0), stop=(kc == 7))
                nc.vector.tensor_add(
                    v4[:, lt, 2 * hp:2 * hp + 2, 0:HD],
                    ot[:].rearrange("p (h c) -> p h c", c=HD),
                    bv_bc[:, hp * 128:(hp + 1) * 128]
                    .rearrange("p (h c) -> p h c", c=HD))

            # ---- attention bursts ----
            GROUPS = [(0, 3, "sA"), (3, 3, "sB"), (6, 3, "sA"),
                      (9, 3, "sB"), (12, 2, "sA"), (14, 2, "sB")]
            KROPE_NT = {0: 0, 3: 1, 6: 2, 9: 2, 12: 3, 14: 3}

            def out_proj(cT, qc, dt_, mq):
                def emit():
                    require("fwo")
                    ops = pp.tile([128, 512], F32, tag="ot", bufs=1,
                                  name=f"op{qc}{dt_}{mq}")
                    for cc in range(4):
                        nc.tensor.matmul(ops[:], lhsT=cT[:, cc, qc, mq, :],
                                         rhs=woT_sb[:, cc, dt_ * 512:(dt_ + 1) * 512],
                                         start=(cc == 0), stop=(cc == 3))
                    o = osp.tile([128, 512], F32, tag="osb", name=f"os{qc}{dt_}{mq}")
                    nc.vector.tensor_copy(o[:], ops[:])
                    dma.dma_start(
                        out=out[qc * 512 + mq * 128: qc * 512 + (mq + 1) * 128,
                                dt_ * 512:(dt_ + 1) * 512],
                        in_=o[:])
                return (f"op{qc}_{dt_}_{mq}", emit)

            cT = ctp.tile([128, 4, QC, 4, 128], DT, tag="cT", bufs=1, name="cT")
            apair_box = [None]

            def burst(h, qc):
                ch2 = h // 2
                prow = slice(64 * (h % 2), 64 * (h % 2) + 64)
                eq, ek = ch2, 4 + ch2
                require(f"r{ch2}_{qc}")
                exp_t = ep.tile([128, KT, 512], DT, tag="exp", name=f"ex{qc}{h}")
                pv = pp.tile([128, 512], F32, tag="pvx", name=f"pv{qc}{h}")
                zero_bank(pv[:])
                pvv = pv[:, 0:320].rearrange("p (q c) -> p q c", c=80)
                if h % 2 == 0:
                    apair_box[0] = app.tile([128, 4, 2, HD], DT, tag="ap",
                                            name=f"ap{qc}{h}")
                apair = apair_box[0]

                def pv_group(kt0, n):
                    for lt in range(kt0, kt0 + n):
                        v_need(lt, ch2)
                    for i in range(n):
                        kt = kt0 + i
                        for qt in range(4):
                            nc.tensor.matmul(
                                pvv[:, qt, 0:65],
                                lhsT=exp_t[:, kt, qt * 128:(qt + 1) * 128],
                                rhs=V_sb[:, kt, h * 65:(h + 1) * 65],
                                start=False, stop=(kt == KT - 1),
                                skip_group_check=True)

                prev = None
                for gi, (kt0, n, tag) in enumerate(GROUPS):
                    require(f"r{4 + ch2}_{KROPE_NT[kt0]}")
                    sg = pp.tile([128, n, 512], F32, tag=tag,
                                 name=f"sg{qc}{h}{kt0}")
                    for i in range(n):
                        kt = kt0 + i
                        nc.tensor.matmul(
                            sg[:, i, :],
                            lhsT=qkT8[prow, ek:9:8 - ek, kt * 128:(kt + 1) * 128],
                            rhs=qkT8[prow, eq:9:8 - eq, qc * 512:(qc + 1) * 512],
                            start=True, stop=True, perf_mode=DR)
                    nc.scalar.activation(
                        exp_t[:, kt0:kt0 + n, :].rearrange("p a b -> p (a b)"),
                        sg[:].rearrange("p a b -> p (a b)"), AF.Exp, scale=SCALE)
                    if gi in (1, 3, 4):
                        if pending:
                            drain(1)
                        else:
                            dummy_fill(3)
                    if prev is not None:
                        pv_group(prev[0], prev[1])
                    prev = (kt0, n)
                pv_group(prev[0], prev[1])

                r = sp.tile([128, 4], F32, tag="rsb", name=f"r{qc}{h}")
                nc.vector.reciprocal(
                    r[:], pvv[:, :, 64:65].rearrange("p q c -> p (q c)"))
                for qt in range(4):
                    nc.vector.tensor_scalar_mul(
                        apair[:, qt, h % 2, :],
                        pvv[:, qt, 0:64], r[:, qt:qt + 1])

                if h % 2 == 1:
                    xp = pp.tile([128, 4, 128], F32, tag="pvx",
                                 name=f"xp{qc}{ch2}")
                    zero_bank(xp[:].rearrange("p a b -> p (a b)"))
                    for qt in range(4):
                        nc.tensor.matmul(
                            xp[:, qt, :],
                            lhsT=apair[:, qt, :, :].rearrange("p a b -> p (a b)"),
                            rhs=ident_sb[:], start=False, stop=True,
                            skip_group_check=True)
                    nc.vector.tensor_copy(cT[:, ch2, qc, :, :], xp[:])
                    if pending:
                        drain(1)

            for hp in range(4):
                for qc in range(QC):
                    burst(2 * hp, qc)
                    burst(2 * hp + 1, qc)
                    if hp == 3 and qc < QC - 1:
                        pending.extend(out_proj(cT, qc, dt_, mq)
                                       for dt_ in range(2) for mq in range(4))
            while pending:
                run_next()
            # flush: qc3 out-proj across all now-idle psum banks in parallel
            require("fwo")
            fA = pp.tile([128, 3, 512], F32, tag="sA", name="fA")
            fB = pp.tile([128, 3, 512], F32, tag="sB", name="fB")
            fO = pp.tile([128, 512], F32, tag="ot", bufs=1, name="fO")
            fP = pp.tile([128, 512], F32, tag="pvx", name="fP")
            banks = [fA[:, 0, :], fA[:, 1, :], fA[:, 2, :],
                     fB[:, 0, :], fB[:, 1, :], fB[:, 2, :], fO[:], fP[:]]
            qc3 = QC - 1
            fl = [(dt_, mq) for dt_ in range(2) for mq in range(4)]
            for (dt_, mq), bank in zip(fl, banks):
                for cc in range(4):
                    nc.tensor.matmul(bank, lhsT=cT[:, cc, qc3, mq, :],
                                     rhs=woT_sb[:, cc, dt_ * 512:(dt_ + 1) * 512],
                                     start=(cc == 0), stop=(cc == 3))
            for (dt_, mq), bank in zip(fl, banks):
                o = osp.tile([128, 512], F32, tag="osb", name=f"fs{dt_}{mq}")
                nc.vector.tensor_copy(o[:], bank)
                dma.dma_start(
                    out=out[qc3 * 512 + mq * 128: qc3 * 512 + (mq + 1) * 128,
                            dt_ * 512:(dt_ + 1) * 512],
                    in_=o[:])
    nc.compile()
    return nc


def _rope_tables_np():
    import ml_dtypes
    bf16 = ml_dtypes.bfloat16
    inv_freq = 1.0 / (10000.0 ** (np.arange(0, HD, 2, dtype=np.float32) / HD))
    t = np.arange(L, dtype=np.float32)
    freqs = np.outer(t, inv_freq).astype(np.float32)       # [L, 32]
    cos_h = np.cos(freqs).T                                # [32, L]
    sin_h = np.sin(freqs).T
    cosT = np.concatenate([cos_h, cos_h], 0)               # [64, L]
    sinT = np.concatenate([-sin_h, sin_h], 0)              # sign baked for rot trick
    return (np.tile(cosT, (2, 1)).astype(bf16),
            np.tile(sinT, (2, 1)).astype(bf16))            # [128, L]


_NC_CACHE = {}


def kernel(x, w_qkv, b_qkv, w_out, b_out):
    import ml_dtypes
    bf16 = ml_dtypes.bfloat16
    f8 = ml_dtypes.float8_e4m3
    if "nc" not in _NC_CACHE:
        _NC_CACHE["nc"] = build_nc()
    nc = _NC_CACHE["nc"]

    cosT, sinT = _rope_tables_np()
    ident = np.eye(128, dtype=np.float32).astype(bf16)
    zeros8 = np.zeros((128, L), dtype=f8)
    p = np.arange(128)
    swap = (p // 64) * 64 + ((p % 64) + 32) % 64
    in_maps = []
    for core in range(8):
        b, g = divmod(core, 2)
        s = slice(512 * g, 512 * (g + 1))
        wqk = np.concatenate([w_qkv[0:D][s], w_qkv[D:2 * D][s]], 0)  # [1024, 1024]
        bqk_v = np.concatenate([b_qkv[0:D][s], b_qkv[D:2 * D][s]])
        # [8, 128, 8, 128] = [fc, dpart, kc, m]
        wqk_t = wqk.T.reshape(8, 128, 8, 128).transpose(2, 1, 0, 3)
        # swapped-row variants of fc0 / fc4 (rotate-half projections);
        # m indexes output features, so permute the last axis
        wswap = np.stack([wqk_t[0][:, :, swap], wqk_t[4][:, :, swap]])
        bqk_cols = np.concatenate(
            [bqk_v.reshape(8, 128).T,
             bqk_v.reshape(8, 128)[[0, 4]].T[swap]], axis=1)   # [128, 10]
        in_maps.append({
            "xT": np.ascontiguousarray(x[b].T).astype(bf16),
            "wqkT": np.ascontiguousarray(
                np.concatenate([wqk_t, wswap], 0)).astype(bf16),
            "wvT": np.ascontiguousarray(w_qkv[2 * D:3 * D][s].T).astype(bf16),
            "bqk": np.ascontiguousarray(bqk_cols).astype(np.float32),
            "bv": b_qkv[2 * D:3 * D][s][None].astype(bf16),
            "woT": np.ascontiguousarray(w_out[:, s].T).astype(bf16),
            "cosT": cosT,
            "sinT": sinT,
            "ident": ident,
            "zeros8": zeros8,
        })
    res = run_bass_kernel_spmd(nc, in_maps, list(range(8)))
    _NC_CACHE["last_results"] = res
    parts = [r["out"] for r in res.results]
    full = np.stack([parts[2 * b] + parts[2 * b + 1] for b in range(4)])
    return (full + b_out[None, None, :]).astype(np.float32)
